# revision 3
# baseline (speedup 1.0000x reference)
"""GCN encoder layer (degree-normalized message passing + BN inference) on 8 Trainium2 cores.

Math (see reference):
    t = X @ W + b                                  [N, H]
    deg = out-degree by src                        [N]
    isd = deg ** -0.5
    nb_sum[i]  = isd[i] * sum_{e: src=i} isd[dst_e] * t[dst_e]
    src_mean   = deg * t            (segment_mean(deg[src]*t[src]) simplifies exactly)
    agg = 0.5*nb_sum + 0.5*src_mean
    out = (agg - mean) * rsqrt(var+eps) * gamma + beta

Strategy (edge-parallel, sharded by src range -> no cross-core reduction):
  - Core c owns src nodes [c*6250, (c+1)*6250); its edges are grouped into
    49 windows of 128 local segments, sorted by dst within a window.
  - Gather X[dst] rows from HBM via gpsimd.dma_gather (512B rows). Indices
    are int16, so the node table is addressed as two views (dst < 32768 and
    dst >= 32768) and each window's edges form a low run then a high run,
    each padded to a multiple of 128 ("batches").
  - Scatter-add via one-hot matmuls on the PE:  Z_T[f, s] += G.T @ O  where
    G = gathered X rows [128 edges, 128 feat] (stationary operand) and
    O[e, s] = (s == src_local[e]) * 0.5*isd[src_e]*isd[dst_e]  (one
    tensor_scalar op: (iota == srcl) * scale per edge-partition).
  - Aggregation commutes with @W:  nb_T = W.T @ Z_T  accumulated in PSUM
    together with the source term  W.T @ (0.5*deg*X_own)_T  and the
    rank-1 bias term b (x) 0.5*deg  (only when b != 0).
  - BN affine is per-partition in the feature-major layout; output written
    feature-major and transposed on the host.
"""

import math
import numpy as np

N_CORES = 8
P = 128
F = 128
H = 128
BN_EPS = 1e-3
SPLIT = 32768      # int16 index limit for dma_gather
CHB = 32           # gather chunk size in batches (<= 64: descriptor ring)

_CACHE = {}


def _wrap16(arr):
    """dma_gather index layout: unwrapped[i] = w[i%16, i//16], replicated x8."""
    w = arr.reshape(-1, 16).T.copy()
    return np.ascontiguousarray(np.tile(w, (8, 1)))


def _build_host_data(edge_pairs, node_features):
    n_nodes = node_features.shape[0]
    src = np.asarray(edge_pairs[:, 0], dtype=np.int64)
    dst = np.asarray(edge_pairs[:, 1], dtype=np.int64)
    deg = np.bincount(src, minlength=n_nodes).astype(np.float64)

    npc = n_nodes // N_CORES
    assert npc * N_CORES == n_nodes
    NW = math.ceil(npc / P)
    npc_pad = NW * P

    core = src // npc
    win = (src - core * npc) // P
    srcl = (src - core * npc) % P
    half = (dst >= SPLIT).astype(np.int64)

    order = np.lexsort((dst, half, win, core))
    dst_s = dst[order]
    core_s, win_s, srcl_s, half_s = core[order], win[order], srcl[order], half[order]
    scale4 = (4.0 * deg[src[order]] * deg[dst_s]).astype(np.float32)

    # counts per (core, window, half)
    cnt = np.zeros((N_CORES, NW, 2), dtype=np.int64)
    np.add.at(cnt, (core_s, win_s, half_s), 1)
    nbL = np.ceil(cnt[:, :, 0].max(axis=0) / P).astype(np.int64)  # [NW]
    nbH = np.ceil(cnt[:, :, 1].max(axis=0) / P).astype(np.int64)
    NBL, NBH = int(nbL.sum()), int(nbH.sum())
    NBtot = NBL + NBH
    cumL = np.concatenate([[0], np.cumsum(nbL)])   # stream-L batch base per window
    cumH = np.concatenate([[0], np.cumsum(nbH)])
    base = np.concatenate([[0], np.cumsum(nbL + nbH)])  # combined batch base

    # run starts in the sorted edge array per (core, window, half)
    flat = cnt.reshape(-1)
    starts_flat = np.concatenate([[0], np.cumsum(flat)[:-1]])
    starts = starts_flat.reshape(N_CORES, NW, 2)

    IDXL = np.zeros((N_CORES, NBL * P), dtype=np.int16)
    IDXH = np.zeros((N_CORES, NBH * P), dtype=np.int16)
    SRCL = np.full((N_CORES, P, NBtot), -1.0, dtype=np.float32)
    SC4 = np.ones((N_CORES, P, NBtot), dtype=np.float32)

    for c in range(N_CORES):
        for w in range(NW):
            for h, (nb_arr, cum, IDX, off) in enumerate(
                    ((nbL, cumL, IDXL, 0), (nbH, cumH, IDXH, SPLIT))):
                nbw = int(nb_arr[w])
                if nbw == 0:
                    continue
                a = starts[c, w, h]
                n = int(cnt[c, w, h])
                nslots = nbw * P
                d_pad = np.zeros(nslots, dtype=np.int16)
                s_pad = np.full(nslots, -1.0, dtype=np.float32)
                c_pad = np.ones(nslots, dtype=np.float32)
                if n > 0:
                    d_pad[:n] = (dst_s[a:a + n] - off).astype(np.int16)
                    d_pad[n:] = d_pad[n - 1] if n > 0 else 0
                    s_pad[:n] = srcl_s[a:a + n]
                    c_pad[:n] = scale4[a:a + n]
                sb = int(cum[w])           # stream batch base
                IDX[c, sb * P:(sb + nbw) * P] = d_pad
                # combined batch position of stream batch k: base[w] (+nbL[w] if high) + k
                cb = int(base[w]) + (int(nbL[w]) if h == 1 else 0)
                # slot i -> (partition i%P, batch i//P)
                SRCL[c, :, cb:cb + nbw] = s_pad.reshape(nbw, P).T
                SC4[c, :, cb:cb + nbw] = c_pad.reshape(nbw, P).T

    XO = np.zeros((N_CORES, npc_pad, F), dtype=np.float32)
    DG2 = np.zeros((N_CORES, P, NW), dtype=np.float32)
    DEGROW = np.zeros((N_CORES, 1, npc_pad), dtype=np.float32)
    nf = np.asarray(node_features, dtype=np.float32)
    for c in range(N_CORES):
        XO[c, :npc] = nf[c * npc:(c + 1) * npc]
        dpad = np.zeros(npc_pad, dtype=np.float32)
        dpad[:npc] = 0.5 * deg[c * npc:(c + 1) * npc]
        DG2[c] = dpad.reshape(NW, P).T
        DEGROW[c, 0] = dpad

    IDXLw = np.stack([_wrap16(IDXL[c]) for c in range(N_CORES)]) if NBL else \
        np.zeros((N_CORES, P, 0), np.int16)
    IDXHw = np.stack([_wrap16(IDXH[c]) for c in range(N_CORES)]) if NBH else \
        np.zeros((N_CORES, P, 0), np.int16)

    return dict(IDXL=IDXLw, IDXH=IDXHw, SRCL=SRCL, SC4=SC4, XO=XO, DG2=DG2,
                DEGROW=DEGROW, NW=NW, NBL=NBL, NBH=NBH, NBtot=NBtot,
                nbL=nbL, nbH=nbH, cumL=cumL, cumH=cumH, base=base,
                npc=npc, npc_pad=npc_pad)


def _build_nc(hd, n_nodes, has_b):
    import concourse.bass as bass
    import concourse.bacc as bacc
    import concourse.mybir as mybir
    import concourse.tile as tile
    from concourse.masks import make_identity

    NW, NBL, NBH, NBtot = hd["NW"], hd["NBL"], hd["NBH"], hd["NBtot"]
    nbL, nbH = hd["nbL"], hd["nbH"]
    cumL, cumH, base = hd["cumL"], hd["cumH"], hd["base"]
    npc_pad = hd["npc_pad"]

    fp32 = mybir.dt.float32
    nc = bacc.Bacc("TRN2", target_bir_lowering=False, debug=False)

    nf_d = nc.dram_tensor("NF", [n_nodes, F], fp32, kind="ExternalInput")
    xo_d = nc.dram_tensor("XO", [npc_pad, F], fp32, kind="ExternalInput")
    il_d = nc.dram_tensor("IDXL", [P, NBL * 8], mybir.dt.int16, kind="ExternalInput")
    ih_d = nc.dram_tensor("IDXH", [P, NBH * 8], mybir.dt.int16, kind="ExternalInput")
    srcl_d = nc.dram_tensor("SRCL", [P, NBtot], fp32, kind="ExternalInput")
    sc4_d = nc.dram_tensor("SC4", [P, NBtot], fp32, kind="ExternalInput")
    dg2_d = nc.dram_tensor("DG2", [P, NW], fp32, kind="ExternalInput")
    iota_d = nc.dram_tensor("IOTA", [P, P], fp32, kind="ExternalInput")
    w_d = nc.dram_tensor("WM", [F, H], fp32, kind="ExternalInput")
    gm_d = nc.dram_tensor("GCOL", [P, 1], fp32, kind="ExternalInput")
    bt_d = nc.dram_tensor("BTCOL", [P, 1], fp32, kind="ExternalInput")
    mm_d = nc.dram_tensor("MMCOL", [P, 1], fp32, kind="ExternalInput")
    mv_d = nc.dram_tensor("MVCOL", [P, 1], fp32, kind="ExternalInput")
    if has_b:
        brow_d = nc.dram_tensor("BROW", [1, H], fp32, kind="ExternalInput")
        degrow_d = nc.dram_tensor("DEGROW", [1, npc_pad], fp32, kind="ExternalInput")
    out_d = nc.dram_tensor("OUT_T", [P, npc_pad], fp32, kind="ExternalOutput")

    with tile.TileContext(nc) as tc:
        with (
            tc.tile_pool(name="meta", bufs=1) as meta,
            tc.tile_pool(name="gl", bufs=3) as glpool,
            tc.tile_pool(name="gh", bufs=2) as ghpool,
            tc.tile_pool(name="o", bufs=4) as opool,
            tc.tile_pool(name="x", bufs=2) as xpool,
            tc.tile_pool(name="z", bufs=2) as zpool,
            tc.tile_pool(name="slab", bufs=1) as slab,
            tc.tile_pool(name="psz", bufs=2, space="PSUM") as psZ,
            tc.tile_pool(name="psnb", bufs=2, space="PSUM") as psNB,
            tc.tile_pool(name="psx", bufs=2, space="PSUM") as psX,
        ):
            il_sb = meta.tile([P, max(NBL, 1) * 8], mybir.dt.int16)
            ih_sb = meta.tile([P, max(NBH, 1) * 8], mybir.dt.int16)
            srcl_sb = meta.tile([P, NBtot], fp32)
            sc4_sb = meta.tile([P, NBtot], fp32)
            scl_sb = meta.tile([P, NBtot], fp32)
            iota_sb = meta.tile([P, P], fp32)
            ident_sb = meta.tile([P, P], fp32)
            w_sb = meta.tile([F, H], fp32)
            dg2_sb = meta.tile([P, NW], fp32)
            gm_sb = meta.tile([P, 1], fp32)
            bt_sb = meta.tile([P, 1], fp32)
            mm_sb = meta.tile([P, 1], fp32)
            mv_sb = meta.tile([P, 1], fp32)
            rs_sb = meta.tile([P, 1], fp32)
            gp_sb = meta.tile([P, 1], fp32)
            bb_sb = meta.tile([P, 1], fp32)

            if NBL:
                nc.sync.dma_start(il_sb[:, :NBL * 8], il_d[:])
            if NBH:
                nc.sync.dma_start(ih_sb[:, :NBH * 8], ih_d[:])
            nc.sync.dma_start(srcl_sb[:], srcl_d[:])
            nc.sync.dma_start(sc4_sb[:], sc4_d[:])
            nc.sync.dma_start(iota_sb[:], iota_d[:])
            nc.sync.dma_start(w_sb[:], w_d[:])
            nc.sync.dma_start(dg2_sb[:], dg2_d[:])
            nc.sync.dma_start(gm_sb[:], gm_d[:])
            nc.sync.dma_start(bt_sb[:], bt_d[:])
            nc.sync.dma_start(mm_sb[:], mm_d[:])
            nc.sync.dma_start(mv_sb[:], mv_d[:])
            make_identity(nc, ident_sb[:])

            # scale' = rsqrt(4*deg_s*deg_d) = 0.5*isd_s*isd_d (Sqrt + exact reciprocal)
            nc.scalar.activation(scl_sb[:], sc4_sb[:], mybir.ActivationFunctionType.Sqrt)
            nc.vector.reciprocal(scl_sb[:], scl_sb[:])

            # BN: g' = gamma * rsqrt(var+eps);  bb = beta - mean*g'
            nc.vector.tensor_scalar(out=rs_sb[:], in0=mv_sb[:], scalar1=BN_EPS,
                                    scalar2=None, op0=mybir.AluOpType.add)
            nc.scalar.activation(rs_sb[:], rs_sb[:], mybir.ActivationFunctionType.Sqrt)
            nc.vector.reciprocal(rs_sb[:], rs_sb[:])
            nc.vector.tensor_tensor(out=gp_sb[:], in0=gm_sb[:], in1=rs_sb[:],
                                    op=mybir.AluOpType.mult)
            nc.vector.tensor_tensor(out=bb_sb[:], in0=mm_sb[:], in1=gp_sb[:],
                                    op=mybir.AluOpType.mult)
            nc.vector.tensor_tensor(out=bb_sb[:], in0=bt_sb[:], in1=bb_sb[:],
                                    op=mybir.AluOpType.subtract)

            if has_b:
                brow_sb = meta.tile([1, H], fp32)
                degrow_sb = meta.tile([1, npc_pad], fp32)
                nc.sync.dma_start(brow_sb[:], brow_d[:])
                nc.sync.dma_start(degrow_sb[:], degrow_d[:])

            outT_sb = slab.tile([P, npc_pad], fp32)

            # ---- gather machinery: two streams (low/high table halves) ----
            streams = {
                "L": dict(nb=NBL, idx=il_sb, view=nf_d[0:min(SPLIT, n_nodes)],
                          pool=glpool, tiles={}),
                "H": dict(nb=NBH, idx=ih_sb, view=(nf_d[SPLIT:n_nodes]
                                                   if n_nodes > SPLIT else None),
                          pool=ghpool, tiles={}),
            }

            def ensure_chunk(s, ci):
                st = streams[s]
                if ci in st["tiles"]:
                    return
                c0, c1 = ci * CHB, min((ci + 1) * CHB, st["nb"])
                nbc = c1 - c0
                gt = st["pool"].tile([P, nbc, F], fp32, tag="g" + s)
                nidx = nbc * P
                nc.gpsimd.dma_gather(
                    gt[:], st["view"], st["idx"][:, c0 * 8:c1 * 8],
                    nidx, nidx, F, single_packet=False)
                st["tiles"][ci] = (c0, gt)

            def gslice(s, j):
                ensure_chunk(s, j // CHB)
                c0, gt = streams[s]["tiles"][j // CHB]
                return gt[:, j - c0, :]

            # ---- main window loop ----
            out_dma_step = max(1, NW // 4)
            for w in range(NW):
                nl, nh = int(nbL[w]), int(nbH[w])
                nbw = nl + nh
                zt = None
                if nbw > 0:
                    psz = psZ.tile([P, P], fp32)
                    for k in range(nbw):
                        j = int(base[w]) + k          # combined batch id
                        if k < nl:
                            g_ap = gslice("L", int(cumL[w]) + k)
                        else:
                            g_ap = gslice("H", int(cumH[w]) + (k - nl))
                        ot = opool.tile([P, P], fp32, tag="o")
                        nc.vector.tensor_scalar(
                            out=ot[:], in0=iota_sb[:],
                            scalar1=srcl_sb[:, j:j + 1], scalar2=scl_sb[:, j:j + 1],
                            op0=mybir.AluOpType.is_equal, op1=mybir.AluOpType.mult,
                        )
                        nc.tensor.matmul(psz[:], lhsT=g_ap, rhs=ot[:],
                                         start=(k == 0), stop=(k == nbw - 1))
                    zt = zpool.tile([P, P], fp32, tag="z")
                    nc.scalar.copy(zt[:], psz[:])

                # source term: (0.5*deg * X_own) transposed
                xt = xpool.tile([P, P], fp32, tag="xt")
                nc.sync.dma_start(xt[:], xo_d[w * P:(w + 1) * P, :])
                xs = xpool.tile([P, P], fp32, tag="xs")
                nc.vector.tensor_scalar(out=xs[:], in0=xt[:],
                                        scalar1=dg2_sb[:, w:w + 1], scalar2=None,
                                        op0=mybir.AluOpType.mult)
                pxt = psX.tile([P, P], fp32)
                nc.tensor.transpose(pxt[:], xs[:], ident_sb[:])
                xT = xpool.tile([P, P], fp32, tag="xT")
                nc.scalar.copy(xT[:], pxt[:])

                psnb = psNB.tile([P, P], fp32)
                first = True
                if zt is not None:
                    nc.tensor.matmul(psnb[:], lhsT=w_sb[:], rhs=zt[:],
                                     start=True, stop=False)
                    first = False
                nc.tensor.matmul(psnb[:], lhsT=w_sb[:], rhs=xT[:],
                                 start=first, stop=not has_b)
                if has_b:
                    nc.tensor.matmul(psnb[:], lhsT=brow_sb[:],
                                     rhs=degrow_sb[:, w * P:(w + 1) * P],
                                     start=False, stop=True)

                # BN affine (per-partition in feature-major layout)
                nc.vector.tensor_scalar(
                    out=outT_sb[:, w * P:(w + 1) * P], in0=psnb[:],
                    scalar1=gp_sb[:], scalar2=bb_sb[:],
                    op0=mybir.AluOpType.mult, op1=mybir.AluOpType.add,
                )

                if (w + 1) % out_dma_step == 0 or w == NW - 1:
                    lo = (w // out_dma_step) * out_dma_step
                    nc.sync.dma_start(out_d[:, lo * P:(w + 1) * P],
                                      outT_sb[:, lo * P:(w + 1) * P])

    nc.compile()
    return nc


def _prepare(edge_pairs, node_features, W, b, gamma, beta, moving_mean, moving_var):
    n_nodes, _ = node_features.shape
    hd = _build_host_data(edge_pairs, node_features)
    has_b = bool(np.any(np.asarray(b) != 0))

    key = (n_nodes, node_features.shape[1], hd["NBtot"],
           tuple(hd["nbL"].tolist()), tuple(hd["nbH"].tolist()), has_b)
    if key not in _CACHE:
        _CACHE.clear()
        _CACHE[key] = _build_nc(hd, n_nodes, has_b)
    nc = _CACHE[key]

    nf = np.ascontiguousarray(np.asarray(node_features, dtype=np.float32))
    iota = np.broadcast_to(np.arange(P, dtype=np.float32), (P, P)).copy()
    in_maps = []
    for c in range(N_CORES):
        m = {
            "NF": nf,
            "XO": np.ascontiguousarray(hd["XO"][c]),
            "IDXL": np.ascontiguousarray(hd["IDXL"][c]),
            "IDXH": np.ascontiguousarray(hd["IDXH"][c]),
            "SRCL": np.ascontiguousarray(hd["SRCL"][c]),
            "SC4": np.ascontiguousarray(hd["SC4"][c]),
            "DG2": np.ascontiguousarray(hd["DG2"][c]),
            "IOTA": iota,
            "WM": np.ascontiguousarray(np.asarray(W, dtype=np.float32)),
            "GCOL": np.asarray(gamma, np.float32).reshape(P, 1).copy(),
            "BTCOL": np.asarray(beta, np.float32).reshape(P, 1).copy(),
            "MMCOL": np.asarray(moving_mean, np.float32).reshape(P, 1).copy(),
            "MVCOL": np.asarray(moving_var, np.float32).reshape(P, 1).copy(),
        }
        if has_b:
            m["BROW"] = np.asarray(b, np.float32).reshape(1, H).copy()
            m["DEGROW"] = np.ascontiguousarray(hd["DEGROW"][c])
        in_maps.append(m)
    return nc, in_maps, hd


def _run(inputs, trace=False):
    from concourse.bass_utils import run_bass_kernel_spmd

    nc, in_maps, hd = _prepare(**inputs)
    res = run_bass_kernel_spmd(nc, in_maps, core_ids=list(range(N_CORES)),
                               trace=trace)
    npc = hd["npc"]
    out = np.empty((npc * N_CORES, H), dtype=np.float32)
    for c in range(N_CORES):
        out[c * npc:(c + 1) * npc] = res.results[c]["OUT_T"].T[:npc]
    return out, res


def kernel(**inputs):
    out, _ = _run(inputs, trace=False)
    return out


def run_traced(**inputs):
    return _run(inputs, trace=True)


# revision 4
# speedup vs baseline: 1.1478x; 1.1478x over previous
"""GCN encoder layer (degree-normalized message passing + BN inference) on 8 Trainium2 cores.

Math (see reference):
    t = X @ W + b                                  [N, H]
    deg = out-degree by src                        [N]
    isd = deg ** -0.5
    nb_sum[i]  = isd[i] * sum_{e: src=i} isd[dst_e] * t[dst_e]
    src_mean   = deg * t            (segment_mean(deg[src]*t[src]) simplifies exactly)
    agg = 0.5*nb_sum + 0.5*src_mean
    out = (agg - mean) * rsqrt(var+eps) * gamma + beta

Strategy (edge-parallel, sharded by src range -> no cross-core reduction):
  - Core c owns src nodes [c*6250, (c+1)*6250); its edges are grouped into
    49 windows of 128 local segments, sorted by dst within a window.
  - Gather X[dst] rows from HBM via gpsimd.dma_gather (512B rows). Indices
    are int16, so the node table is addressed as two views (dst < 32768 and
    dst >= 32768) and each window's edges form a low run then a high run,
    each padded to a multiple of 128 ("batches").
  - Scatter-add via one-hot matmuls on the PE:  Z_T[f, s] += G.T @ O  where
    G = gathered X rows [128 edges, 128 feat] (stationary operand) and
    O[e, s] = (s == src_local[e]) * 0.5*isd[src_e]*isd[dst_e]  (one
    tensor_scalar op: (iota == srcl) * scale per edge-partition).
  - Aggregation commutes with @W:  nb_T = W.T @ Z_T  accumulated in PSUM
    together with the source term  W.T @ (0.5*deg*X_own)_T  and the
    rank-1 bias term b (x) 0.5*deg  (only when b != 0).
  - BN affine is per-partition in the feature-major layout; output written
    feature-major and transposed on the host.
"""

import math
import numpy as np

N_CORES = 8
P = 128
F = 128
H = 128
BN_EPS = 1e-3
SPLIT = 32768      # int16 index limit for dma_gather
CHB = 32           # gather chunk size in batches (<= 64: descriptor ring)

_CACHE = {}


def _wrap16(arr):
    """dma_gather index layout: unwrapped[i] = w[i%16, i//16], replicated x8."""
    w = arr.reshape(-1, 16).T.copy()
    return np.ascontiguousarray(np.tile(w, (8, 1)))


def _build_host_data(edge_pairs, node_features):
    n_nodes = node_features.shape[0]
    src = np.asarray(edge_pairs[:, 0], dtype=np.int64)
    dst = np.asarray(edge_pairs[:, 1], dtype=np.int64)
    deg = np.bincount(src, minlength=n_nodes).astype(np.float64)

    npc = n_nodes // N_CORES
    assert npc * N_CORES == n_nodes
    NW = math.ceil(npc / P)
    npc_pad = NW * P

    core = src // npc
    win = (src - core * npc) // P
    srcl = (src - core * npc) % P
    half = (dst >= SPLIT).astype(np.int64)

    order = np.lexsort((dst, half, win, core))
    dst_s = dst[order]
    core_s, win_s, srcl_s, half_s = core[order], win[order], srcl[order], half[order]
    scale4 = (4.0 * deg[src[order]] * deg[dst_s]).astype(np.float32)

    # counts per (core, window, half)
    cnt = np.zeros((N_CORES, NW, 2), dtype=np.int64)
    np.add.at(cnt, (core_s, win_s, half_s), 1)
    nbL = np.ceil(cnt[:, :, 0].max(axis=0) / P).astype(np.int64)  # [NW]
    nbH = np.ceil(cnt[:, :, 1].max(axis=0) / P).astype(np.int64)
    NBL, NBH = int(nbL.sum()), int(nbH.sum())
    NBtot = NBL + NBH
    cumL = np.concatenate([[0], np.cumsum(nbL)])   # stream-L batch base per window
    cumH = np.concatenate([[0], np.cumsum(nbH)])
    base = np.concatenate([[0], np.cumsum(nbL + nbH)])  # combined batch base

    # run starts in the sorted edge array per (core, window, half)
    flat = cnt.reshape(-1)
    starts_flat = np.concatenate([[0], np.cumsum(flat)[:-1]])
    starts = starts_flat.reshape(N_CORES, NW, 2)

    IDXL = np.zeros((N_CORES, NBL * P), dtype=np.int16)
    IDXH = np.zeros((N_CORES, NBH * P), dtype=np.int16)
    SRCL = np.full((N_CORES, P, NBtot), -1.0, dtype=np.float32)
    SC4 = np.ones((N_CORES, P, NBtot), dtype=np.float32)

    for c in range(N_CORES):
        for w in range(NW):
            for h, (nb_arr, cum, IDX, off) in enumerate(
                    ((nbL, cumL, IDXL, 0), (nbH, cumH, IDXH, SPLIT))):
                nbw = int(nb_arr[w])
                if nbw == 0:
                    continue
                a = starts[c, w, h]
                n = int(cnt[c, w, h])
                nslots = nbw * P
                d_pad = np.zeros(nslots, dtype=np.int16)
                s_pad = np.full(nslots, -1.0, dtype=np.float32)
                c_pad = np.ones(nslots, dtype=np.float32)
                if n > 0:
                    d_pad[:n] = (dst_s[a:a + n] - off).astype(np.int16)
                    d_pad[n:] = d_pad[n - 1] if n > 0 else 0
                    s_pad[:n] = srcl_s[a:a + n]
                    c_pad[:n] = scale4[a:a + n]
                sb = int(cum[w])           # stream batch base
                IDX[c, sb * P:(sb + nbw) * P] = d_pad
                # combined batch position of stream batch k: base[w] (+nbL[w] if high) + k
                cb = int(base[w]) + (int(nbL[w]) if h == 1 else 0)
                # slot i -> (partition i%P, batch i//P)
                SRCL[c, :, cb:cb + nbw] = s_pad.reshape(nbw, P).T
                SC4[c, :, cb:cb + nbw] = c_pad.reshape(nbw, P).T

    XO = np.zeros((N_CORES, npc_pad, F), dtype=np.float32)
    DG2 = np.zeros((N_CORES, P, NW), dtype=np.float32)
    DEGROW = np.zeros((N_CORES, 1, npc_pad), dtype=np.float32)
    nf = np.asarray(node_features, dtype=np.float32)
    for c in range(N_CORES):
        XO[c, :npc] = nf[c * npc:(c + 1) * npc]
        dpad = np.zeros(npc_pad, dtype=np.float32)
        dpad[:npc] = 0.5 * deg[c * npc:(c + 1) * npc]
        DG2[c] = dpad.reshape(NW, P).T
        DEGROW[c, 0] = dpad

    IDXLw = np.stack([_wrap16(IDXL[c]) for c in range(N_CORES)]) if NBL else \
        np.zeros((N_CORES, P, 0), np.int16)
    IDXHw = np.stack([_wrap16(IDXH[c]) for c in range(N_CORES)]) if NBH else \
        np.zeros((N_CORES, P, 0), np.int16)

    return dict(IDXL=IDXLw, IDXH=IDXHw, SRCL=SRCL, SC4=SC4, XO=XO, DG2=DG2,
                DEGROW=DEGROW, NW=NW, NBL=NBL, NBH=NBH, NBtot=NBtot,
                nbL=nbL, nbH=nbH, cumL=cumL, cumH=cumH, base=base,
                npc=npc, npc_pad=npc_pad)


def _build_nc(hd, n_nodes, has_b):
    import concourse.bass as bass
    import concourse.bacc as bacc
    import concourse.mybir as mybir
    import concourse.tile as tile
    from concourse.masks import make_identity

    NW, NBL, NBH, NBtot = hd["NW"], hd["NBL"], hd["NBH"], hd["NBtot"]
    nbL, nbH = hd["nbL"], hd["nbH"]
    cumL, cumH, base = hd["cumL"], hd["cumH"], hd["base"]
    npc_pad = hd["npc_pad"]

    fp32 = mybir.dt.float32
    nc = bacc.Bacc("TRN2", target_bir_lowering=False, debug=False,
                   num_swdge_queues=4)

    nf_d = nc.dram_tensor("NF", [n_nodes, F], fp32, kind="ExternalInput")
    xo_d = nc.dram_tensor("XO", [npc_pad, F], fp32, kind="ExternalInput")
    il_d = nc.dram_tensor("IDXL", [P, NBL * 8], mybir.dt.int16, kind="ExternalInput")
    ih_d = nc.dram_tensor("IDXH", [P, NBH * 8], mybir.dt.int16, kind="ExternalInput")
    srcl_d = nc.dram_tensor("SRCL", [P, NBtot], fp32, kind="ExternalInput")
    sc4_d = nc.dram_tensor("SC4", [P, NBtot], fp32, kind="ExternalInput")
    dg2_d = nc.dram_tensor("DG2", [P, NW], fp32, kind="ExternalInput")
    iota_d = nc.dram_tensor("IOTA", [P, P], fp32, kind="ExternalInput")
    w_d = nc.dram_tensor("WM", [F, H], fp32, kind="ExternalInput")
    gm_d = nc.dram_tensor("GCOL", [P, 1], fp32, kind="ExternalInput")
    bt_d = nc.dram_tensor("BTCOL", [P, 1], fp32, kind="ExternalInput")
    mm_d = nc.dram_tensor("MMCOL", [P, 1], fp32, kind="ExternalInput")
    mv_d = nc.dram_tensor("MVCOL", [P, 1], fp32, kind="ExternalInput")
    if has_b:
        brow_d = nc.dram_tensor("BROW", [1, H], fp32, kind="ExternalInput")
        degrow_d = nc.dram_tensor("DEGROW", [1, npc_pad], fp32, kind="ExternalInput")
    out_d = nc.dram_tensor("OUT_T", [P, npc_pad], fp32, kind="ExternalOutput")

    with tile.TileContext(nc) as tc:
        with (
            tc.tile_pool(name="meta", bufs=1) as meta,
            tc.tile_pool(name="gl", bufs=3) as glpool,
            tc.tile_pool(name="gh", bufs=2) as ghpool,
            tc.tile_pool(name="o", bufs=4) as opool,
            tc.tile_pool(name="x", bufs=2) as xpool,
            tc.tile_pool(name="z", bufs=2) as zpool,
            tc.tile_pool(name="slab", bufs=1) as slab,
            tc.tile_pool(name="psz", bufs=2, space="PSUM") as psZ,
            tc.tile_pool(name="psnb", bufs=2, space="PSUM") as psNB,
            tc.tile_pool(name="psx", bufs=2, space="PSUM") as psX,
        ):
            il_sb = meta.tile([P, max(NBL, 1) * 8], mybir.dt.int16)
            ih_sb = meta.tile([P, max(NBH, 1) * 8], mybir.dt.int16)
            srcl_sb = meta.tile([P, NBtot], fp32)
            sc4_sb = meta.tile([P, NBtot], fp32)
            scl_sb = meta.tile([P, NBtot], fp32)
            iota_sb = meta.tile([P, P], fp32)
            ident_sb = meta.tile([P, P], fp32)
            w_sb = meta.tile([F, H], fp32)
            dg2_sb = meta.tile([P, NW], fp32)
            gm_sb = meta.tile([P, 1], fp32)
            bt_sb = meta.tile([P, 1], fp32)
            mm_sb = meta.tile([P, 1], fp32)
            mv_sb = meta.tile([P, 1], fp32)
            rs_sb = meta.tile([P, 1], fp32)
            gp_sb = meta.tile([P, 1], fp32)
            bb_sb = meta.tile([P, 1], fp32)

            if NBL:
                nc.sync.dma_start(il_sb[:, :NBL * 8], il_d[:])
            if NBH:
                nc.sync.dma_start(ih_sb[:, :NBH * 8], ih_d[:])
            nc.sync.dma_start(srcl_sb[:], srcl_d[:])
            nc.sync.dma_start(sc4_sb[:], sc4_d[:])
            nc.sync.dma_start(iota_sb[:], iota_d[:])
            nc.sync.dma_start(w_sb[:], w_d[:])
            nc.sync.dma_start(dg2_sb[:], dg2_d[:])
            nc.sync.dma_start(gm_sb[:], gm_d[:])
            nc.sync.dma_start(bt_sb[:], bt_d[:])
            nc.sync.dma_start(mm_sb[:], mm_d[:])
            nc.sync.dma_start(mv_sb[:], mv_d[:])
            make_identity(nc, ident_sb[:])

            # scale' = rsqrt(4*deg_s*deg_d) = 0.5*isd_s*isd_d (Sqrt + exact reciprocal)
            nc.scalar.activation(scl_sb[:], sc4_sb[:], mybir.ActivationFunctionType.Sqrt)
            nc.vector.reciprocal(scl_sb[:], scl_sb[:])

            # BN: g' = gamma * rsqrt(var+eps);  bb = beta - mean*g'
            nc.vector.tensor_scalar(out=rs_sb[:], in0=mv_sb[:], scalar1=BN_EPS,
                                    scalar2=None, op0=mybir.AluOpType.add)
            nc.scalar.activation(rs_sb[:], rs_sb[:], mybir.ActivationFunctionType.Sqrt)
            nc.vector.reciprocal(rs_sb[:], rs_sb[:])
            nc.vector.tensor_tensor(out=gp_sb[:], in0=gm_sb[:], in1=rs_sb[:],
                                    op=mybir.AluOpType.mult)
            nc.vector.tensor_tensor(out=bb_sb[:], in0=mm_sb[:], in1=gp_sb[:],
                                    op=mybir.AluOpType.mult)
            nc.vector.tensor_tensor(out=bb_sb[:], in0=bt_sb[:], in1=bb_sb[:],
                                    op=mybir.AluOpType.subtract)

            if has_b:
                brow_sb = meta.tile([1, H], fp32)
                degrow_sb = meta.tile([1, npc_pad], fp32)
                nc.sync.dma_start(brow_sb[:], brow_d[:])
                nc.sync.dma_start(degrow_sb[:], degrow_d[:])

            outT_sb = slab.tile([P, npc_pad], fp32)

            # ---- gather machinery: two streams (low/high table halves) ----
            qrr = [0]
            streams = {
                "L": dict(nb=NBL, idx=il_sb, view=nf_d[0:min(SPLIT, n_nodes)],
                          pool=glpool, tiles={}),
                "H": dict(nb=NBH, idx=ih_sb, view=(nf_d[SPLIT:n_nodes]
                                                   if n_nodes > SPLIT else None),
                          pool=ghpool, tiles={}),
            }

            def ensure_chunk(s, ci):
                st = streams[s]
                if ci in st["tiles"]:
                    return
                c0, c1 = ci * CHB, min((ci + 1) * CHB, st["nb"])
                nbc = c1 - c0
                gt = st["pool"].tile([P, nbc, F], fp32, tag="g" + s)
                nidx = nbc * P
                nc.gpsimd.dma_gather(
                    gt[:], st["view"], st["idx"][:, c0 * 8:c1 * 8],
                    nidx, nidx, F, single_packet=False,
                    queue_num=qrr[0] % 4)
                qrr[0] += 1
                st["tiles"][ci] = (c0, gt)

            def gslice(s, j):
                ensure_chunk(s, j // CHB)
                c0, gt = streams[s]["tiles"][j // CHB]
                return gt[:, j - c0, :]

            # ---- main window loop ----
            out_dma_step = max(1, NW // 4)
            for w in range(NW):
                nl, nh = int(nbL[w]), int(nbH[w])
                nbw = nl + nh
                zt = None
                if nbw > 0:
                    psz = psZ.tile([P, P], fp32)
                    for k in range(nbw):
                        j = int(base[w]) + k          # combined batch id
                        if k < nl:
                            g_ap = gslice("L", int(cumL[w]) + k)
                        else:
                            g_ap = gslice("H", int(cumH[w]) + (k - nl))
                        ot = opool.tile([P, P], fp32, tag="o")
                        nc.vector.tensor_scalar(
                            out=ot[:], in0=iota_sb[:],
                            scalar1=srcl_sb[:, j:j + 1], scalar2=scl_sb[:, j:j + 1],
                            op0=mybir.AluOpType.is_equal, op1=mybir.AluOpType.mult,
                        )
                        nc.tensor.matmul(psz[:], lhsT=g_ap, rhs=ot[:],
                                         start=(k == 0), stop=(k == nbw - 1))
                    zt = zpool.tile([P, P], fp32, tag="z")
                    nc.scalar.copy(zt[:], psz[:])

                # source term: (0.5*deg * X_own) transposed
                xt = xpool.tile([P, P], fp32, tag="xt")
                nc.sync.dma_start(xt[:], xo_d[w * P:(w + 1) * P, :])
                xs = xpool.tile([P, P], fp32, tag="xs")
                nc.vector.tensor_scalar(out=xs[:], in0=xt[:],
                                        scalar1=dg2_sb[:, w:w + 1], scalar2=None,
                                        op0=mybir.AluOpType.mult)
                pxt = psX.tile([P, P], fp32)
                nc.tensor.transpose(pxt[:], xs[:], ident_sb[:])
                xT = xpool.tile([P, P], fp32, tag="xT")
                nc.scalar.copy(xT[:], pxt[:])

                psnb = psNB.tile([P, P], fp32)
                first = True
                if zt is not None:
                    nc.tensor.matmul(psnb[:], lhsT=w_sb[:], rhs=zt[:],
                                     start=True, stop=False)
                    first = False
                nc.tensor.matmul(psnb[:], lhsT=w_sb[:], rhs=xT[:],
                                 start=first, stop=not has_b)
                if has_b:
                    nc.tensor.matmul(psnb[:], lhsT=brow_sb[:],
                                     rhs=degrow_sb[:, w * P:(w + 1) * P],
                                     start=False, stop=True)

                # BN affine (per-partition in feature-major layout)
                nc.vector.tensor_scalar(
                    out=outT_sb[:, w * P:(w + 1) * P], in0=psnb[:],
                    scalar1=gp_sb[:], scalar2=bb_sb[:],
                    op0=mybir.AluOpType.mult, op1=mybir.AluOpType.add,
                )

                if (w + 1) % out_dma_step == 0 or w == NW - 1:
                    lo = (w // out_dma_step) * out_dma_step
                    nc.sync.dma_start(out_d[:, lo * P:(w + 1) * P],
                                      outT_sb[:, lo * P:(w + 1) * P])

    nc.compile()
    return nc


def _prepare(edge_pairs, node_features, W, b, gamma, beta, moving_mean, moving_var):
    n_nodes, _ = node_features.shape
    hd = _build_host_data(edge_pairs, node_features)
    has_b = bool(np.any(np.asarray(b) != 0))

    key = (n_nodes, node_features.shape[1], hd["NBtot"],
           tuple(hd["nbL"].tolist()), tuple(hd["nbH"].tolist()), has_b)
    if key not in _CACHE:
        _CACHE.clear()
        _CACHE[key] = _build_nc(hd, n_nodes, has_b)
    nc = _CACHE[key]

    nf = np.ascontiguousarray(np.asarray(node_features, dtype=np.float32))
    iota = np.broadcast_to(np.arange(P, dtype=np.float32), (P, P)).copy()
    in_maps = []
    for c in range(N_CORES):
        m = {
            "NF": nf,
            "XO": np.ascontiguousarray(hd["XO"][c]),
            "IDXL": np.ascontiguousarray(hd["IDXL"][c]),
            "IDXH": np.ascontiguousarray(hd["IDXH"][c]),
            "SRCL": np.ascontiguousarray(hd["SRCL"][c]),
            "SC4": np.ascontiguousarray(hd["SC4"][c]),
            "DG2": np.ascontiguousarray(hd["DG2"][c]),
            "IOTA": iota,
            "WM": np.ascontiguousarray(np.asarray(W, dtype=np.float32)),
            "GCOL": np.asarray(gamma, np.float32).reshape(P, 1).copy(),
            "BTCOL": np.asarray(beta, np.float32).reshape(P, 1).copy(),
            "MMCOL": np.asarray(moving_mean, np.float32).reshape(P, 1).copy(),
            "MVCOL": np.asarray(moving_var, np.float32).reshape(P, 1).copy(),
        }
        if has_b:
            m["BROW"] = np.asarray(b, np.float32).reshape(1, H).copy()
            m["DEGROW"] = np.ascontiguousarray(hd["DEGROW"][c])
        in_maps.append(m)
    return nc, in_maps, hd


def _run(inputs, trace=False):
    from concourse.bass_utils import run_bass_kernel_spmd

    nc, in_maps, hd = _prepare(**inputs)
    res = run_bass_kernel_spmd(nc, in_maps, core_ids=list(range(N_CORES)),
                               trace=trace)
    npc = hd["npc"]
    out = np.empty((npc * N_CORES, H), dtype=np.float32)
    for c in range(N_CORES):
        out[c * npc:(c + 1) * npc] = res.results[c]["OUT_T"].T[:npc]
    return out, res


def kernel(**inputs):
    out, _ = _run(inputs, trace=False)
    return out


def run_traced(**inputs):
    return _run(inputs, trace=True)


# revision 5
# speedup vs baseline: 1.2971x; 1.1301x over previous
"""GCN encoder layer (degree-normalized message passing + BN inference) on 8 Trainium2 cores.

Math (see reference):
    t = X @ W + b                                  [N, H]
    deg = out-degree by src                        [N]
    isd = deg ** -0.5
    nb_sum[i]  = isd[i] * sum_{e: src=i} isd[dst_e] * t[dst_e]
    src_mean   = deg * t            (segment_mean(deg[src]*t[src]) simplifies exactly)
    agg = 0.5*nb_sum + 0.5*src_mean
    out = (agg - mean) * rsqrt(var+eps) * gamma + beta

Strategy (edge-parallel, sharded by src range -> no cross-core reduction):
  - Core c owns src nodes [c*6250, (c+1)*6250); its edges are grouped into
    49 windows of 128 local segments, sorted by dst within a window.
  - Gather X[dst] rows from HBM via gpsimd.dma_gather (512B rows). Indices
    are int16, so the node table is addressed as two views (dst < 32768 and
    dst >= 32768) and each window's edges form a low run then a high run,
    each padded to a multiple of 128 ("batches").
  - Scatter-add via one-hot matmuls on the PE:  Z_T[f, s] += G.T @ O  where
    G = gathered X rows [128 edges, 128 feat] (stationary operand) and
    O[e, s] = (s == src_local[e]) * 0.5*isd[src_e]*isd[dst_e]  (one
    tensor_scalar op: (iota == srcl) * scale per edge-partition).
  - Aggregation commutes with @W:  nb_T = W.T @ Z_T  accumulated in PSUM
    together with the source term  W.T @ (0.5*deg*X_own)_T  and the
    rank-1 bias term b (x) 0.5*deg  (only when b != 0).
  - BN affine is per-partition in the feature-major layout; output written
    feature-major and transposed on the host.
"""

import math
import numpy as np

N_CORES = 8
P = 128
F = 128
H = 128
BN_EPS = 1e-3
SPLIT = 32768      # int16 index limit for dma_gather
CHB = 16           # gather chunk size in batches

_CACHE = {}


def _wrap16(arr):
    """dma_gather index layout: unwrapped[i] = w[i%16, i//16], replicated x8."""
    w = arr.reshape(-1, 16).T.copy()
    return np.ascontiguousarray(np.tile(w, (8, 1)))


def _build_host_data(edge_pairs, node_features):
    n_nodes = node_features.shape[0]
    src = np.asarray(edge_pairs[:, 0], dtype=np.int64)
    dst = np.asarray(edge_pairs[:, 1], dtype=np.int64)
    deg = np.bincount(src, minlength=n_nodes).astype(np.float64)

    npc = n_nodes // N_CORES
    assert npc * N_CORES == n_nodes
    NW = math.ceil(npc / P)
    npc_pad = NW * P

    core = src // npc
    win = (src - core * npc) // P
    srcl = (src - core * npc) % P
    half = (dst >= SPLIT).astype(np.int64)

    order = np.lexsort((dst, half, win, core))
    dst_s = dst[order]
    core_s, win_s, srcl_s, half_s = core[order], win[order], srcl[order], half[order]
    scale4 = (4.0 * deg[src[order]] * deg[dst_s]).astype(np.float32)

    # counts per (core, window, half)
    cnt = np.zeros((N_CORES, NW, 2), dtype=np.int64)
    np.add.at(cnt, (core_s, win_s, half_s), 1)
    nbL = np.ceil(cnt[:, :, 0].max(axis=0) / P).astype(np.int64)  # [NW]
    nbH = np.ceil(cnt[:, :, 1].max(axis=0) / P).astype(np.int64)
    NBL, NBH = int(nbL.sum()), int(nbH.sum())
    NBtot = NBL + NBH
    cumL = np.concatenate([[0], np.cumsum(nbL)])   # stream-L batch base per window
    cumH = np.concatenate([[0], np.cumsum(nbH)])
    base = np.concatenate([[0], np.cumsum(nbL + nbH)])  # combined batch base

    # run starts in the sorted edge array per (core, window, half)
    flat = cnt.reshape(-1)
    starts_flat = np.concatenate([[0], np.cumsum(flat)[:-1]])
    starts = starts_flat.reshape(N_CORES, NW, 2)

    IDXL = np.zeros((N_CORES, NBL * P), dtype=np.int16)
    IDXH = np.zeros((N_CORES, NBH * P), dtype=np.int16)
    SRCL = np.full((N_CORES, P, NBtot), -1.0, dtype=np.float32)
    SC4 = np.ones((N_CORES, P, NBtot), dtype=np.float32)

    for c in range(N_CORES):
        for w in range(NW):
            for h, (nb_arr, cum, IDX, off) in enumerate(
                    ((nbL, cumL, IDXL, 0), (nbH, cumH, IDXH, SPLIT))):
                nbw = int(nb_arr[w])
                if nbw == 0:
                    continue
                a = starts[c, w, h]
                n = int(cnt[c, w, h])
                nslots = nbw * P
                d_pad = np.zeros(nslots, dtype=np.int16)
                s_pad = np.full(nslots, -1.0, dtype=np.float32)
                c_pad = np.ones(nslots, dtype=np.float32)
                if n > 0:
                    d_pad[:n] = (dst_s[a:a + n] - off).astype(np.int16)
                    d_pad[n:] = d_pad[n - 1] if n > 0 else 0
                    s_pad[:n] = srcl_s[a:a + n]
                    c_pad[:n] = scale4[a:a + n]
                sb = int(cum[w])           # stream batch base
                IDX[c, sb * P:(sb + nbw) * P] = d_pad
                # combined batch position of stream batch k: base[w] (+nbL[w] if high) + k
                cb = int(base[w]) + (int(nbL[w]) if h == 1 else 0)
                # slot i -> (partition i%P, batch i//P)
                SRCL[c, :, cb:cb + nbw] = s_pad.reshape(nbw, P).T
                SC4[c, :, cb:cb + nbw] = c_pad.reshape(nbw, P).T

    XO = np.zeros((N_CORES, npc_pad, F), dtype=np.float32)
    DG2 = np.zeros((N_CORES, P, NW), dtype=np.float32)
    DEGROW = np.zeros((N_CORES, 1, npc_pad), dtype=np.float32)
    nf = np.asarray(node_features, dtype=np.float32)
    for c in range(N_CORES):
        XO[c, :npc] = nf[c * npc:(c + 1) * npc]
        dpad = np.zeros(npc_pad, dtype=np.float32)
        dpad[:npc] = 0.5 * deg[c * npc:(c + 1) * npc]
        DG2[c] = dpad.reshape(NW, P).T
        DEGROW[c, 0] = dpad

    IDXLw = np.stack([_wrap16(IDXL[c]) for c in range(N_CORES)]) if NBL else \
        np.zeros((N_CORES, P, 0), np.int16)
    IDXHw = np.stack([_wrap16(IDXH[c]) for c in range(N_CORES)]) if NBH else \
        np.zeros((N_CORES, P, 0), np.int16)

    return dict(IDXL=IDXLw, IDXH=IDXHw, SRCL=SRCL, SC4=SC4, XO=XO, DG2=DG2,
                DEGROW=DEGROW, NW=NW, NBL=NBL, NBH=NBH, NBtot=NBtot,
                nbL=nbL, nbH=nbH, cumL=cumL, cumH=cumH, base=base,
                npc=npc, npc_pad=npc_pad)


def _build_nc(hd, n_nodes, has_b):
    import concourse.bass as bass
    import concourse.bacc as bacc
    import concourse.mybir as mybir
    import concourse.tile as tile

    NW, NBL, NBH, NBtot = hd["NW"], hd["NBL"], hd["NBH"], hd["NBtot"]
    nbL, nbH = hd["nbL"], hd["nbH"]
    cumL, cumH, base = hd["cumL"], hd["cumH"], hd["base"]
    npc_pad = hd["npc_pad"]

    fp32 = mybir.dt.float32
    nc = bacc.Bacc("TRN2", target_bir_lowering=False, debug=False,
                   num_swdge_queues=4)

    nf_d = nc.dram_tensor("NF", [n_nodes, F], fp32, kind="ExternalInput")
    xo_d = nc.dram_tensor("XO", [npc_pad, F], fp32, kind="ExternalInput")
    il_d = nc.dram_tensor("IDXL", [P, NBL * 8], mybir.dt.int16, kind="ExternalInput")
    ih_d = nc.dram_tensor("IDXH", [P, NBH * 8], mybir.dt.int16, kind="ExternalInput")
    srcl_d = nc.dram_tensor("SRCL", [P, NBtot], fp32, kind="ExternalInput")
    sc4_d = nc.dram_tensor("SC4", [P, NBtot], fp32, kind="ExternalInput")
    dg2_d = nc.dram_tensor("DG2", [P, NW], fp32, kind="ExternalInput")
    iota_d = nc.dram_tensor("IOTA", [P, P], fp32, kind="ExternalInput")
    ident_d = nc.dram_tensor("IDENT", [P, P], fp32, kind="ExternalInput")
    w_d = nc.dram_tensor("WM", [F, H], fp32, kind="ExternalInput")
    gm_d = nc.dram_tensor("GCOL", [P, 1], fp32, kind="ExternalInput")
    bt_d = nc.dram_tensor("BTCOL", [P, 1], fp32, kind="ExternalInput")
    mm_d = nc.dram_tensor("MMCOL", [P, 1], fp32, kind="ExternalInput")
    mv_d = nc.dram_tensor("MVCOL", [P, 1], fp32, kind="ExternalInput")
    if has_b:
        brow_d = nc.dram_tensor("BROW", [1, H], fp32, kind="ExternalInput")
        degrow_d = nc.dram_tensor("DEGROW", [1, npc_pad], fp32, kind="ExternalInput")
    out_d = nc.dram_tensor("OUT_T", [P, npc_pad], fp32, kind="ExternalOutput")

    with tile.TileContext(nc) as tc:
        with (
            tc.tile_pool(name="meta", bufs=1) as meta,
            tc.tile_pool(name="gl", bufs=7) as glpool,
            tc.tile_pool(name="gh", bufs=5) as ghpool,
            tc.tile_pool(name="o", bufs=4) as opool,
            tc.tile_pool(name="x", bufs=2) as xpool,
            tc.tile_pool(name="z", bufs=2) as zpool,
            tc.tile_pool(name="slab", bufs=1) as slab,
            tc.tile_pool(name="psz", bufs=2, space="PSUM") as psZ,
            tc.tile_pool(name="psnb", bufs=2, space="PSUM") as psNB,
            tc.tile_pool(name="psx", bufs=2, space="PSUM") as psX,
        ):
            il_sb = meta.tile([P, max(NBL, 1) * 8], mybir.dt.int16)
            ih_sb = meta.tile([P, max(NBH, 1) * 8], mybir.dt.int16)
            srcl_sb = meta.tile([P, NBtot], fp32)
            sc4_sb = meta.tile([P, NBtot], fp32)
            scl_sb = meta.tile([P, NBtot], fp32)
            iota_sb = meta.tile([P, P], fp32)
            ident_sb = meta.tile([P, P], fp32)
            w_sb = meta.tile([F, H], fp32)
            dg2_sb = meta.tile([P, NW], fp32)
            gm_sb = meta.tile([P, 1], fp32)
            bt_sb = meta.tile([P, 1], fp32)
            mm_sb = meta.tile([P, 1], fp32)
            mv_sb = meta.tile([P, 1], fp32)
            rs_sb = meta.tile([P, 1], fp32)
            gp_sb = meta.tile([P, 1], fp32)
            bb_sb = meta.tile([P, 1], fp32)

            if NBL:
                nc.sync.dma_start(il_sb[:, :NBL * 8], il_d[:])
            if NBH:
                nc.sync.dma_start(ih_sb[:, :NBH * 8], ih_d[:])
            nc.sync.dma_start(srcl_sb[:], srcl_d[:])
            nc.sync.dma_start(sc4_sb[:], sc4_d[:])
            nc.sync.dma_start(iota_sb[:], iota_d[:])
            nc.sync.dma_start(w_sb[:], w_d[:])
            nc.sync.dma_start(dg2_sb[:], dg2_d[:])
            nc.sync.dma_start(gm_sb[:], gm_d[:])
            nc.sync.dma_start(bt_sb[:], bt_d[:])
            nc.sync.dma_start(mm_sb[:], mm_d[:])
            nc.sync.dma_start(mv_sb[:], mv_d[:])
            nc.sync.dma_start(ident_sb[:], ident_d[:])

            # scale' = rsqrt(4*deg_s*deg_d) = 0.5*isd_s*isd_d (Sqrt + exact reciprocal)
            nc.scalar.activation(scl_sb[:], sc4_sb[:], mybir.ActivationFunctionType.Sqrt)
            nc.vector.reciprocal(scl_sb[:], scl_sb[:])

            # BN: g' = gamma * rsqrt(var+eps);  bb = beta - mean*g'
            nc.vector.tensor_scalar(out=rs_sb[:], in0=mv_sb[:], scalar1=BN_EPS,
                                    scalar2=None, op0=mybir.AluOpType.add)
            nc.scalar.activation(rs_sb[:], rs_sb[:], mybir.ActivationFunctionType.Sqrt)
            nc.vector.reciprocal(rs_sb[:], rs_sb[:])
            nc.vector.tensor_tensor(out=gp_sb[:], in0=gm_sb[:], in1=rs_sb[:],
                                    op=mybir.AluOpType.mult)
            nc.vector.tensor_tensor(out=bb_sb[:], in0=mm_sb[:], in1=gp_sb[:],
                                    op=mybir.AluOpType.mult)
            nc.vector.tensor_tensor(out=bb_sb[:], in0=bt_sb[:], in1=bb_sb[:],
                                    op=mybir.AluOpType.subtract)

            if has_b:
                brow_sb = meta.tile([1, H], fp32)
                degrow_sb = meta.tile([1, npc_pad], fp32)
                nc.sync.dma_start(brow_sb[:], brow_d[:])
                nc.sync.dma_start(degrow_sb[:], degrow_d[:])

            outT_sb = slab.tile([P, npc_pad], fp32)

            # ---- gather machinery: two streams (low/high table halves) ----
            streams = {
                "L": dict(nb=NBL, idx=il_sb, view=nf_d[0:min(SPLIT, n_nodes)],
                          pool=glpool, tiles={}),
                "H": dict(nb=NBH, idx=ih_sb, view=(nf_d[SPLIT:n_nodes]
                                                   if n_nodes > SPLIT else None),
                          pool=ghpool, tiles={}),
            }

            # emit all gathers up front, in consumption (first-use) order
            chunk_order, seen = [], set()
            for w in range(NW):
                for k in range(int(nbL[w]) + int(nbH[w])):
                    if k < int(nbL[w]):
                        key = ("L", (int(cumL[w]) + k) // CHB)
                    else:
                        key = ("H", (int(cumH[w]) + k - int(nbL[w])) // CHB)
                    if key not in seen:
                        seen.add(key)
                        chunk_order.append(key)
            for sname, ci in chunk_order:
                st = streams[sname]
                c0, c1 = ci * CHB, min((ci + 1) * CHB, st["nb"])
                nbc = c1 - c0
                gt = st["pool"].tile([P, nbc, F], fp32, tag="g" + sname)
                nidx = nbc * P
                nc.gpsimd.dma_gather(
                    gt[:], st["view"], st["idx"][:, c0 * 8:c1 * 8],
                    nidx, nidx, F, single_packet=False, queue_num=0)
                st["tiles"][ci] = (c0, gt)

            def gslice(s, j):
                c0, gt = streams[s]["tiles"][j // CHB]
                return gt[:, j - c0, :]

            # ---- main window loop ----
            out_dma_step = max(1, NW // 4)
            for w in range(NW):
                nl, nh = int(nbL[w]), int(nbH[w])
                nbw = nl + nh
                zt = None
                if nbw > 0:
                    psz = psZ.tile([P, P], fp32)
                    for k in range(nbw):
                        j = int(base[w]) + k          # combined batch id
                        if k < nl:
                            g_ap = gslice("L", int(cumL[w]) + k)
                        else:
                            g_ap = gslice("H", int(cumH[w]) + (k - nl))
                        ot = opool.tile([P, P], fp32, tag="o")
                        nc.vector.tensor_scalar(
                            out=ot[:], in0=iota_sb[:],
                            scalar1=srcl_sb[:, j:j + 1], scalar2=scl_sb[:, j:j + 1],
                            op0=mybir.AluOpType.is_equal, op1=mybir.AluOpType.mult,
                        )
                        nc.tensor.matmul(psz[:], lhsT=g_ap, rhs=ot[:],
                                         start=(k == 0), stop=(k == nbw - 1))
                    zt = zpool.tile([P, P], fp32, tag="z")
                    nc.scalar.copy(zt[:], psz[:])

                # source term: (0.5*deg * X_own) transposed
                xt = xpool.tile([P, P], fp32, tag="xt")
                nc.sync.dma_start(xt[:], xo_d[w * P:(w + 1) * P, :])
                xs = xpool.tile([P, P], fp32, tag="xs")
                nc.vector.tensor_scalar(out=xs[:], in0=xt[:],
                                        scalar1=dg2_sb[:, w:w + 1], scalar2=None,
                                        op0=mybir.AluOpType.mult)
                pxt = psX.tile([P, P], fp32)
                nc.tensor.transpose(pxt[:], xs[:], ident_sb[:])
                xT = xpool.tile([P, P], fp32, tag="xT")
                nc.scalar.copy(xT[:], pxt[:])

                psnb = psNB.tile([P, P], fp32)
                first = True
                if zt is not None:
                    nc.tensor.matmul(psnb[:], lhsT=w_sb[:], rhs=zt[:],
                                     start=True, stop=False)
                    first = False
                nc.tensor.matmul(psnb[:], lhsT=w_sb[:], rhs=xT[:],
                                 start=first, stop=not has_b)
                if has_b:
                    nc.tensor.matmul(psnb[:], lhsT=brow_sb[:],
                                     rhs=degrow_sb[:, w * P:(w + 1) * P],
                                     start=False, stop=True)

                # BN affine (per-partition in feature-major layout)
                nc.vector.tensor_scalar(
                    out=outT_sb[:, w * P:(w + 1) * P], in0=psnb[:],
                    scalar1=gp_sb[:], scalar2=bb_sb[:],
                    op0=mybir.AluOpType.mult, op1=mybir.AluOpType.add,
                )

                if (w + 1) % out_dma_step == 0 or w == NW - 1:
                    lo = (w // out_dma_step) * out_dma_step
                    nc.sync.dma_start(out_d[:, lo * P:(w + 1) * P],
                                      outT_sb[:, lo * P:(w + 1) * P])

    # SWDGE queue ownership: each DMASW sem lane is owned by one queue, so
    # set queue_num = lane % num_queues after Tile assigned lanes.
    from concourse.tile_scheduler import PROC_NAME_TO_IDX
    idx_to_proc = {v: k for k, v in PROC_NAME_TO_IDX.items()}
    for bb in nc.main_func.blocks:
        for ins in bb.instructions:
            if isinstance(ins, mybir.InstDMAGatherAnt):
                proc = idx_to_proc.get(ins.bass_scheduled_proc, "")
                if proc.startswith("DMASW"):
                    ins.queue_num = int(proc[5:]) % 4

    nc.compile()
    return nc


def _prepare(edge_pairs, node_features, W, b, gamma, beta, moving_mean, moving_var):
    n_nodes, _ = node_features.shape
    hd = _build_host_data(edge_pairs, node_features)
    has_b = bool(np.any(np.asarray(b) != 0))

    key = (n_nodes, node_features.shape[1], hd["NBtot"],
           tuple(hd["nbL"].tolist()), tuple(hd["nbH"].tolist()), has_b)
    if key not in _CACHE:
        _CACHE.clear()
        _CACHE[key] = _build_nc(hd, n_nodes, has_b)
    nc = _CACHE[key]

    nf = np.ascontiguousarray(np.asarray(node_features, dtype=np.float32))
    iota = np.broadcast_to(np.arange(P, dtype=np.float32), (P, P)).copy()
    in_maps = []
    for c in range(N_CORES):
        m = {
            "NF": nf,
            "XO": np.ascontiguousarray(hd["XO"][c]),
            "IDXL": np.ascontiguousarray(hd["IDXL"][c]),
            "IDXH": np.ascontiguousarray(hd["IDXH"][c]),
            "SRCL": np.ascontiguousarray(hd["SRCL"][c]),
            "SC4": np.ascontiguousarray(hd["SC4"][c]),
            "DG2": np.ascontiguousarray(hd["DG2"][c]),
            "IOTA": iota,
            "IDENT": np.eye(P, dtype=np.float32),
            "WM": np.ascontiguousarray(np.asarray(W, dtype=np.float32)),
            "GCOL": np.asarray(gamma, np.float32).reshape(P, 1).copy(),
            "BTCOL": np.asarray(beta, np.float32).reshape(P, 1).copy(),
            "MMCOL": np.asarray(moving_mean, np.float32).reshape(P, 1).copy(),
            "MVCOL": np.asarray(moving_var, np.float32).reshape(P, 1).copy(),
        }
        if has_b:
            m["BROW"] = np.asarray(b, np.float32).reshape(1, H).copy()
            m["DEGROW"] = np.ascontiguousarray(hd["DEGROW"][c])
        in_maps.append(m)
    return nc, in_maps, hd


def _run(inputs, trace=False):
    from concourse.bass_utils import run_bass_kernel_spmd

    nc, in_maps, hd = _prepare(**inputs)
    res = run_bass_kernel_spmd(nc, in_maps, core_ids=list(range(N_CORES)),
                               trace=trace)
    npc = hd["npc"]
    out = np.empty((npc * N_CORES, H), dtype=np.float32)
    for c in range(N_CORES):
        out[c * npc:(c + 1) * npc] = res.results[c]["OUT_T"].T[:npc]
    return out, res


def kernel(**inputs):
    out, _ = _run(inputs, trace=False)
    return out


def run_traced(**inputs):
    return _run(inputs, trace=True)


# revision 7
# speedup vs baseline: 1.3255x; 1.0219x over previous
"""GCN encoder layer (degree-normalized message passing + BN inference) on 8 Trainium2 cores.

Math (see reference):
    t = X @ W + b                                  [N, H]
    deg = out-degree by src                        [N]
    isd = deg ** -0.5
    nb_sum[i]  = isd[i] * sum_{e: src=i} isd[dst_e] * t[dst_e]
    src_mean   = deg * t            (segment_mean(deg[src]*t[src]) simplifies exactly)
    agg = 0.5*nb_sum + 0.5*src_mean
    out = (agg - mean) * rsqrt(var+eps) * gamma + beta

Strategy (edge-parallel, sharded by src range -> no cross-core reduction):
  - Core c owns src nodes [c*6250, (c+1)*6250); its edges are grouped into
    49 windows of 128 local segments, sorted by dst within a window.
  - Gather X[dst] rows from HBM via gpsimd.dma_gather (512B rows). Indices
    are int16, so the node table is addressed as two views (dst < 32768 and
    dst >= 32768) and each window's edges form a low run then a high run,
    each padded to a multiple of 128 ("batches").
  - Scatter-add via one-hot matmuls on the PE:  Z_T[f, s] += G.T @ O  where
    G = gathered X rows [128 edges, 128 feat] (stationary operand) and
    O[e, s] = (s == src_local[e]) * 0.5*isd[src_e]*isd[dst_e]  (one
    tensor_scalar op: (iota == srcl) * scale per edge-partition).
  - Aggregation commutes with @W:  nb_T = W.T @ Z_T  accumulated in PSUM
    together with the source term  W.T @ (0.5*deg*X_own)_T  and the
    rank-1 bias term b (x) 0.5*deg  (only when b != 0).
  - BN affine is per-partition in the feature-major layout; output written
    feature-major and transposed on the host.
"""

import math
import numpy as np

N_CORES = 8
P = 128
F = 128
H = 128
BN_EPS = 1e-3
SPLIT = 32768      # int16 index limit for dma_gather
CHB = 16           # gather chunk size in batches

_CACHE = {}


def _wrap16(arr):
    """dma_gather index layout: unwrapped[i] = w[i%16, i//16], replicated x8."""
    w = arr.reshape(-1, 16).T.copy()
    return np.ascontiguousarray(np.tile(w, (8, 1)))


def _build_host_data(edge_pairs, node_features):
    n_nodes = node_features.shape[0]
    src = np.asarray(edge_pairs[:, 0], dtype=np.int64)
    dst = np.asarray(edge_pairs[:, 1], dtype=np.int64)
    deg = np.bincount(src, minlength=n_nodes).astype(np.float64)

    npc = n_nodes // N_CORES
    assert npc * N_CORES == n_nodes
    NW = math.ceil(npc / P)
    npc_pad = NW * P

    core = src // npc
    win = (src - core * npc) // P
    srcl = (src - core * npc) % P
    half = (dst >= SPLIT).astype(np.int64)

    order = np.lexsort((dst, half, win, core))
    dst_s = dst[order]
    core_s, win_s, srcl_s, half_s = core[order], win[order], srcl[order], half[order]
    scale4 = (4.0 * deg[src[order]] * deg[dst_s]).astype(np.float32)

    # counts per (core, window, half)
    cnt = np.zeros((N_CORES, NW, 2), dtype=np.int64)
    np.add.at(cnt, (core_s, win_s, half_s), 1)
    nbL = np.ceil(cnt[:, :, 0].max(axis=0) / P).astype(np.int64)  # [NW]
    nbH = np.ceil(cnt[:, :, 1].max(axis=0) / P).astype(np.int64)
    NBL, NBH = int(nbL.sum()), int(nbH.sum())
    NBtot = NBL + NBH
    cumL = np.concatenate([[0], np.cumsum(nbL)])   # stream-L batch base per window
    cumH = np.concatenate([[0], np.cumsum(nbH)])
    base = np.concatenate([[0], np.cumsum(nbL + nbH)])  # combined batch base

    # run starts in the sorted edge array per (core, window, half)
    flat = cnt.reshape(-1)
    starts_flat = np.concatenate([[0], np.cumsum(flat)[:-1]])
    starts = starts_flat.reshape(N_CORES, NW, 2)

    IDXL = np.zeros((N_CORES, NBL * P), dtype=np.int16)
    IDXH = np.zeros((N_CORES, NBH * P), dtype=np.int16)
    SRCL = np.full((N_CORES, P, NBtot), -1.0, dtype=np.float32)
    SC4 = np.ones((N_CORES, P, NBtot), dtype=np.float32)

    for c in range(N_CORES):
        for w in range(NW):
            for h, (nb_arr, cum, IDX, off) in enumerate(
                    ((nbL, cumL, IDXL, 0), (nbH, cumH, IDXH, SPLIT))):
                nbw = int(nb_arr[w])
                if nbw == 0:
                    continue
                a = starts[c, w, h]
                n = int(cnt[c, w, h])
                nslots = nbw * P
                d_pad = np.zeros(nslots, dtype=np.int16)
                s_pad = np.full(nslots, -1.0, dtype=np.float32)
                c_pad = np.ones(nslots, dtype=np.float32)
                if n > 0:
                    d_pad[:n] = (dst_s[a:a + n] - off).astype(np.int16)
                    d_pad[n:] = d_pad[n - 1] if n > 0 else 0
                    s_pad[:n] = srcl_s[a:a + n]
                    c_pad[:n] = scale4[a:a + n]
                sb = int(cum[w])           # stream batch base
                IDX[c, sb * P:(sb + nbw) * P] = d_pad
                # combined batch position of stream batch k: base[w] (+nbL[w] if high) + k
                cb = int(base[w]) + (int(nbL[w]) if h == 1 else 0)
                # slot i -> (partition i%P, batch i//P)
                SRCL[c, :, cb:cb + nbw] = s_pad.reshape(nbw, P).T
                SC4[c, :, cb:cb + nbw] = c_pad.reshape(nbw, P).T

    XO = np.zeros((N_CORES, npc_pad, F), dtype=np.float32)
    DG2 = np.zeros((N_CORES, P, NW), dtype=np.float32)
    DEGROW = np.zeros((N_CORES, 1, npc_pad), dtype=np.float32)
    nf = np.asarray(node_features, dtype=np.float32)
    for c in range(N_CORES):
        XO[c, :npc] = nf[c * npc:(c + 1) * npc]
        dpad = np.zeros(npc_pad, dtype=np.float32)
        dpad[:npc] = 0.5 * deg[c * npc:(c + 1) * npc]
        DG2[c] = dpad.reshape(NW, P).T
        DEGROW[c, 0] = dpad

    IDXLw = np.stack([_wrap16(IDXL[c]) for c in range(N_CORES)]) if NBL else \
        np.zeros((N_CORES, P, 0), np.int16)
    IDXHw = np.stack([_wrap16(IDXH[c]) for c in range(N_CORES)]) if NBH else \
        np.zeros((N_CORES, P, 0), np.int16)

    return dict(IDXL=IDXLw, IDXH=IDXHw, SRCL=SRCL, SC4=SC4, XO=XO, DG2=DG2,
                DEGROW=DEGROW, NW=NW, NBL=NBL, NBH=NBH, NBtot=NBtot,
                nbL=nbL, nbH=nbH, cumL=cumL, cumH=cumH, base=base,
                npc=npc, npc_pad=npc_pad)


def _build_nc(hd, n_nodes, has_b):
    import concourse.bass as bass
    import concourse.bacc as bacc
    import concourse.mybir as mybir
    import concourse.tile as tile

    NW, NBL, NBH, NBtot = hd["NW"], hd["NBL"], hd["NBH"], hd["NBtot"]
    nbL, nbH = hd["nbL"], hd["nbH"]
    cumL, cumH, base = hd["cumL"], hd["cumH"], hd["base"]
    npc_pad = hd["npc_pad"]

    fp32 = mybir.dt.float32
    nc = bacc.Bacc("TRN2", target_bir_lowering=False, debug=False,
                   num_swdge_queues=4)

    nf_d = nc.dram_tensor("NF", [n_nodes, F], fp32, kind="ExternalInput")
    xo_d = nc.dram_tensor("XO", [npc_pad, F], fp32, kind="ExternalInput")
    il_d = nc.dram_tensor("IDXL", [P, NBL * 8], mybir.dt.int16, kind="ExternalInput")
    ih_d = nc.dram_tensor("IDXH", [P, NBH * 8], mybir.dt.int16, kind="ExternalInput")
    srcl_d = nc.dram_tensor("SRCL", [P, NBtot], fp32, kind="ExternalInput")
    sc4_d = nc.dram_tensor("SC4", [P, NBtot], fp32, kind="ExternalInput")
    dg2_d = nc.dram_tensor("DG2", [P, NW], fp32, kind="ExternalInput")
    iota_d = nc.dram_tensor("IOTA", [P, P], fp32, kind="ExternalInput")
    ident_d = nc.dram_tensor("IDENT", [P, P], fp32, kind="ExternalInput")
    w_d = nc.dram_tensor("WM", [F, H], fp32, kind="ExternalInput")
    gm_d = nc.dram_tensor("GCOL", [P, 1], fp32, kind="ExternalInput")
    bt_d = nc.dram_tensor("BTCOL", [P, 1], fp32, kind="ExternalInput")
    mm_d = nc.dram_tensor("MMCOL", [P, 1], fp32, kind="ExternalInput")
    mv_d = nc.dram_tensor("MVCOL", [P, 1], fp32, kind="ExternalInput")
    if has_b:
        brow_d = nc.dram_tensor("BROW", [1, H], fp32, kind="ExternalInput")
        degrow_d = nc.dram_tensor("DEGROW", [1, npc_pad], fp32, kind="ExternalInput")
    out_d = nc.dram_tensor("OUT_T", [P, npc_pad], fp32, kind="ExternalOutput")

    with tile.TileContext(nc) as tc:
        with (
            tc.tile_pool(name="meta", bufs=1) as meta,
            tc.tile_pool(name="gl", bufs=7) as glpool,
            tc.tile_pool(name="gh", bufs=5) as ghpool,
            tc.tile_pool(name="o", bufs=4) as opool,
            tc.tile_pool(name="x", bufs=2) as xpool,
            tc.tile_pool(name="z", bufs=2) as zpool,
            tc.tile_pool(name="slab", bufs=1) as slab,
            tc.tile_pool(name="psz", bufs=2, space="PSUM") as psZ,
            tc.tile_pool(name="psnb", bufs=2, space="PSUM") as psNB,
            tc.tile_pool(name="psx", bufs=2, space="PSUM") as psX,
        ):
            il_sb = meta.tile([P, max(NBL, 1) * 8], mybir.dt.int16)
            ih_sb = meta.tile([P, max(NBH, 1) * 8], mybir.dt.int16)
            srcl_sb = meta.tile([P, NBtot], fp32)
            sc4_sb = meta.tile([P, NBtot], fp32)
            scl_sb = meta.tile([P, NBtot], fp32)
            iota_sb = meta.tile([P, P], fp32)
            ident_sb = meta.tile([P, P], fp32)
            w_sb = meta.tile([F, H], fp32)
            dg2_sb = meta.tile([P, NW], fp32)
            gm_sb = meta.tile([P, 1], fp32)
            bt_sb = meta.tile([P, 1], fp32)
            mm_sb = meta.tile([P, 1], fp32)
            mv_sb = meta.tile([P, 1], fp32)
            rs_sb = meta.tile([P, 1], fp32)
            gp_sb = meta.tile([P, 1], fp32)
            bb_sb = meta.tile([P, 1], fp32)

            if NBL:
                nc.sync.dma_start(il_sb[:, :NBL * 8], il_d[:])
            if NBH:
                nc.sync.dma_start(ih_sb[:, :NBH * 8], ih_d[:])
            nc.sync.dma_start(srcl_sb[:], srcl_d[:])
            nc.sync.dma_start(sc4_sb[:], sc4_d[:])
            nc.sync.dma_start(iota_sb[:], iota_d[:])
            nc.sync.dma_start(w_sb[:], w_d[:])
            nc.sync.dma_start(dg2_sb[:], dg2_d[:])
            nc.sync.dma_start(gm_sb[:], gm_d[:])
            nc.sync.dma_start(bt_sb[:], bt_d[:])
            nc.sync.dma_start(mm_sb[:], mm_d[:])
            nc.sync.dma_start(mv_sb[:], mv_d[:])
            nc.sync.dma_start(ident_sb[:], ident_d[:])

            # scale' = rsqrt(4*deg_s*deg_d) = 0.5*isd_s*isd_d (Sqrt + exact reciprocal)
            nc.scalar.activation(scl_sb[:], sc4_sb[:], mybir.ActivationFunctionType.Sqrt)
            nc.vector.reciprocal(scl_sb[:], scl_sb[:])

            # BN: g' = gamma * rsqrt(var+eps);  bb = beta - mean*g'
            nc.vector.tensor_scalar(out=rs_sb[:], in0=mv_sb[:], scalar1=BN_EPS,
                                    scalar2=None, op0=mybir.AluOpType.add)
            nc.scalar.activation(rs_sb[:], rs_sb[:], mybir.ActivationFunctionType.Sqrt)
            nc.vector.reciprocal(rs_sb[:], rs_sb[:])
            nc.vector.tensor_tensor(out=gp_sb[:], in0=gm_sb[:], in1=rs_sb[:],
                                    op=mybir.AluOpType.mult)
            nc.vector.tensor_tensor(out=bb_sb[:], in0=mm_sb[:], in1=gp_sb[:],
                                    op=mybir.AluOpType.mult)
            nc.vector.tensor_tensor(out=bb_sb[:], in0=bt_sb[:], in1=bb_sb[:],
                                    op=mybir.AluOpType.subtract)

            if has_b:
                brow_sb = meta.tile([1, H], fp32)
                degrow_sb = meta.tile([1, npc_pad], fp32)
                nc.sync.dma_start(brow_sb[:], brow_d[:])
                nc.sync.dma_start(degrow_sb[:], degrow_d[:])

            outT_sb = slab.tile([P, npc_pad], fp32)

            # ---- gather machinery: two streams (low/high table halves) ----
            streams = {
                "L": dict(nb=NBL, idx=il_sb, view=nf_d[0:min(SPLIT, n_nodes)],
                          pool=glpool, tiles={}),
                "H": dict(nb=NBH, idx=ih_sb, view=(nf_d[SPLIT:n_nodes]
                                                   if n_nodes > SPLIT else None),
                          pool=ghpool, tiles={}),
            }

            # emit all gathers up front, in consumption (first-use) order
            chunk_order, seen = [], set()
            for w in range(NW):
                for k in range(int(nbL[w]) + int(nbH[w])):
                    if k < int(nbL[w]):
                        key = ("L", (int(cumL[w]) + k) // CHB)
                    else:
                        key = ("H", (int(cumH[w]) + k - int(nbL[w])) // CHB)
                    if key not in seen:
                        seen.add(key)
                        chunk_order.append(key)
            for sname, ci in chunk_order:
                st = streams[sname]
                c0, c1 = ci * CHB, min((ci + 1) * CHB, st["nb"])
                nbc = c1 - c0
                gt = st["pool"].tile([P, nbc, F], fp32, tag="g" + sname)
                nidx = nbc * P
                nc.gpsimd.dma_gather(
                    gt[:], st["view"], st["idx"][:, c0 * 8:c1 * 8],
                    nidx, nidx, F, single_packet=False, queue_num=0)
                st["tiles"][ci] = (c0, gt)

            def gslice(s, j):
                c0, gt = streams[s]["tiles"][j // CHB]
                return gt[:, j - c0, :]

            # ---- main window loop ----
            out_dma_step = max(1, NW // 4)
            for w in range(NW):
                nl, nh = int(nbL[w]), int(nbH[w])
                nbw = nl + nh
                zt = None
                if nbw > 0:
                    psz = psZ.tile([P, P], fp32)
                    for k in range(nbw):
                        j = int(base[w]) + k          # combined batch id
                        if k < nl:
                            g_ap = gslice("L", int(cumL[w]) + k)
                        else:
                            g_ap = gslice("H", int(cumH[w]) + (k - nl))
                        ot = opool.tile([P, P], fp32, tag="o")
                        nc.vector.tensor_scalar(
                            out=ot[:], in0=iota_sb[:],
                            scalar1=srcl_sb[:, j:j + 1], scalar2=scl_sb[:, j:j + 1],
                            op0=mybir.AluOpType.is_equal, op1=mybir.AluOpType.mult,
                        )
                        nc.tensor.matmul(psz[:], lhsT=g_ap, rhs=ot[:],
                                         start=(k == 0), stop=(k == nbw - 1))
                    zt = zpool.tile([P, P], fp32, tag="z")
                    nc.scalar.copy(zt[:], psz[:])

                # source term: (0.5*deg * X_own) transposed
                xt = xpool.tile([P, P], fp32, tag="xt")
                nc.sync.dma_start(xt[:], xo_d[w * P:(w + 1) * P, :])
                xs = xpool.tile([P, P], fp32, tag="xs")
                nc.scalar.activation(xs[:], xt[:],
                                     mybir.ActivationFunctionType.Identity,
                                     scale=dg2_sb[:, w:w + 1])
                pxt = psX.tile([P, P], fp32)
                nc.tensor.transpose(pxt[:], xs[:], ident_sb[:])
                xT = xpool.tile([P, P], fp32, tag="xT")
                nc.scalar.copy(xT[:], pxt[:])

                psnb = psNB.tile([P, P], fp32)
                first = True
                if zt is not None:
                    nc.tensor.matmul(psnb[:], lhsT=w_sb[:], rhs=zt[:],
                                     start=True, stop=False)
                    first = False
                nc.tensor.matmul(psnb[:], lhsT=w_sb[:], rhs=xT[:],
                                 start=first, stop=not has_b)
                if has_b:
                    nc.tensor.matmul(psnb[:], lhsT=brow_sb[:],
                                     rhs=degrow_sb[:, w * P:(w + 1) * P],
                                     start=False, stop=True)

                # BN affine (per-partition in feature-major layout)
                nc.scalar.activation(
                    outT_sb[:, w * P:(w + 1) * P], psnb[:],
                    mybir.ActivationFunctionType.Identity,
                    bias=bb_sb[:], scale=gp_sb[:],
                )

                if (w + 1) % out_dma_step == 0 or w == NW - 1:
                    lo = (w // out_dma_step) * out_dma_step
                    nc.sync.dma_start(out_d[:, lo * P:(w + 1) * P],
                                      outT_sb[:, lo * P:(w + 1) * P])

    # SWDGE queue ownership: each DMASW sem lane is owned by one queue, so
    # set queue_num = lane % num_queues after Tile assigned lanes.
    from concourse.tile_scheduler import PROC_NAME_TO_IDX
    idx_to_proc = {v: k for k, v in PROC_NAME_TO_IDX.items()}
    for bb in nc.main_func.blocks:
        for ins in bb.instructions:
            if isinstance(ins, mybir.InstDMAGatherAnt):
                proc = idx_to_proc.get(ins.bass_scheduled_proc, "")
                if proc.startswith("DMASW"):
                    ins.queue_num = int(proc[5:]) % 4

    nc.compile()
    return nc


def _prepare(edge_pairs, node_features, W, b, gamma, beta, moving_mean, moving_var):
    n_nodes, _ = node_features.shape
    hd = _build_host_data(edge_pairs, node_features)
    has_b = bool(np.any(np.asarray(b) != 0))

    key = (n_nodes, node_features.shape[1], hd["NBtot"],
           tuple(hd["nbL"].tolist()), tuple(hd["nbH"].tolist()), has_b)
    if key not in _CACHE:
        _CACHE.clear()
        _CACHE[key] = _build_nc(hd, n_nodes, has_b)
    nc = _CACHE[key]

    nf = np.ascontiguousarray(np.asarray(node_features, dtype=np.float32))
    iota = np.broadcast_to(np.arange(P, dtype=np.float32), (P, P)).copy()
    in_maps = []
    for c in range(N_CORES):
        m = {
            "NF": nf,
            "XO": np.ascontiguousarray(hd["XO"][c]),
            "IDXL": np.ascontiguousarray(hd["IDXL"][c]),
            "IDXH": np.ascontiguousarray(hd["IDXH"][c]),
            "SRCL": np.ascontiguousarray(hd["SRCL"][c]),
            "SC4": np.ascontiguousarray(hd["SC4"][c]),
            "DG2": np.ascontiguousarray(hd["DG2"][c]),
            "IOTA": iota,
            "IDENT": np.eye(P, dtype=np.float32),
            "WM": np.ascontiguousarray(np.asarray(W, dtype=np.float32)),
            "GCOL": np.asarray(gamma, np.float32).reshape(P, 1).copy(),
            "BTCOL": np.asarray(beta, np.float32).reshape(P, 1).copy(),
            "MMCOL": np.asarray(moving_mean, np.float32).reshape(P, 1).copy(),
            "MVCOL": np.asarray(moving_var, np.float32).reshape(P, 1).copy(),
        }
        if has_b:
            m["BROW"] = np.asarray(b, np.float32).reshape(1, H).copy()
            m["DEGROW"] = np.ascontiguousarray(hd["DEGROW"][c])
        in_maps.append(m)
    return nc, in_maps, hd


def _run(inputs, trace=False):
    from concourse.bass_utils import run_bass_kernel_spmd

    nc, in_maps, hd = _prepare(**inputs)
    res = run_bass_kernel_spmd(nc, in_maps, core_ids=list(range(N_CORES)),
                               trace=trace)
    npc = hd["npc"]
    out = np.empty((npc * N_CORES, H), dtype=np.float32)
    for c in range(N_CORES):
        out[c * npc:(c + 1) * npc] = res.results[c]["OUT_T"].T[:npc]
    return out, res


def kernel(**inputs):
    out, _ = _run(inputs, trace=False)
    return out


def run_traced(**inputs):
    return _run(inputs, trace=True)


# revision 11
# speedup vs baseline: 3.3144x; 2.5005x over previous
"""GCN encoder layer (degree-normalized message passing + BN inference) on 8 Trainium2 cores.

Math (see reference):
    t = X @ W + b                                  [N, H]
    deg = out-degree by src                        [N]
    isd = deg ** -0.5
    nb_sum[i]  = isd[i] * sum_{e: src=i} isd[dst_e] * t[dst_e]
    src_mean   = deg * t            (segment_mean(deg[src]*t[src]) simplifies exactly)
    agg = 0.5*nb_sum + 0.5*src_mean
    out = (agg - mean) * rsqrt(var+eps) * gamma + beta

Strategy (edge-parallel, sharded by src range -> no cross-core reduction):
  - Core c owns src nodes [c*6250, (c+1)*6250); its edges are grouped into
    49 windows of 128 local segments, sorted by dst within a window.
  - Gather X[dst] rows from HBM via gpsimd.dma_gather (512B rows). Indices
    are int16, so the node table is addressed as two views (dst < 32768 and
    dst >= 32768) and each window's edges form a low run then a high run,
    each padded to a multiple of 128 ("batches").
  - Scatter-add via one-hot matmuls on the PE:  Z_T[f, s] += G.T @ O  where
    G = gathered X rows [128 edges, 128 feat] (stationary operand) and
    O[e, s] = (s == src_local[e]) * 0.5*isd[src_e]*isd[dst_e]  (one
    tensor_scalar op: (iota == srcl) * scale per edge-partition).
  - Aggregation commutes with @W:  nb_T = W.T @ Z_T  accumulated in PSUM
    together with the source term  W.T @ (0.5*deg*X_own)_T  and the
    rank-1 bias term b (x) 0.5*deg  (only when b != 0).
  - BN affine is per-partition in the feature-major layout; output written
    feature-major and transposed on the host.
"""

import math
import numpy as np

N_CORES = 8
P = 128
F = 128
H = 128
BN_EPS = 1e-3
SPLIT = 32768      # int16 index limit for dma_gather
CHB = 16           # gather chunk size in batches

_CACHE = {}


def _wrap16(arr):
    """dma_gather index layout: unwrapped[i] = w[i%16, i//16], replicated x8."""
    w = arr.reshape(-1, 16).T.copy()
    return np.ascontiguousarray(np.tile(w, (8, 1)))


def _build_host_data(edge_pairs, node_features):
    n_nodes = node_features.shape[0]
    src = np.asarray(edge_pairs[:, 0], dtype=np.int64)
    dst = np.asarray(edge_pairs[:, 1], dtype=np.int64)
    deg = np.bincount(src, minlength=n_nodes).astype(np.float64)

    npc = n_nodes // N_CORES
    assert npc * N_CORES == n_nodes
    NW = math.ceil(npc / P)
    npc_pad = NW * P

    core = src // npc
    win = (src - core * npc) // P
    srcl = (src - core * npc) % P
    half = (dst >= SPLIT).astype(np.int64)

    order = np.lexsort((dst, half, win, core))
    dst_s = dst[order]
    core_s, win_s, srcl_s, half_s = core[order], win[order], srcl[order], half[order]
    scale4 = (4.0 * deg[src[order]] * deg[dst_s]).astype(np.float32)

    # counts per (core, window, half)
    cnt = np.zeros((N_CORES, NW, 2), dtype=np.int64)
    np.add.at(cnt, (core_s, win_s, half_s), 1)
    nbL = np.ceil(cnt[:, :, 0].max(axis=0) / P).astype(np.int64)  # [NW]
    nbH = np.ceil(cnt[:, :, 1].max(axis=0) / P).astype(np.int64)
    NBL, NBH = int(nbL.sum()), int(nbH.sum())
    NBtot = NBL + NBH
    cumL = np.concatenate([[0], np.cumsum(nbL)])   # stream-L batch base per window
    cumH = np.concatenate([[0], np.cumsum(nbH)])
    base = np.concatenate([[0], np.cumsum(nbL + nbH)])  # combined batch base

    # run starts in the sorted edge array per (core, window, half)
    flat = cnt.reshape(-1)
    starts_flat = np.concatenate([[0], np.cumsum(flat)[:-1]])
    starts = starts_flat.reshape(N_CORES, NW, 2)

    IDXL = np.zeros((N_CORES, NBL * P), dtype=np.int16)
    IDXH = np.zeros((N_CORES, NBH * P), dtype=np.int16)
    SRCL = np.full((N_CORES, P, NBtot), -1.0, dtype=np.float32)
    SC4 = np.ones((N_CORES, P, NBtot), dtype=np.float32)

    for c in range(N_CORES):
        for w in range(NW):
            for h, (nb_arr, cum, IDX, off) in enumerate(
                    ((nbL, cumL, IDXL, 0), (nbH, cumH, IDXH, SPLIT))):
                nbw = int(nb_arr[w])
                if nbw == 0:
                    continue
                a = starts[c, w, h]
                n = int(cnt[c, w, h])
                nslots = nbw * P
                d_pad = np.zeros(nslots, dtype=np.int16)
                s_pad = np.full(nslots, -1.0, dtype=np.float32)
                c_pad = np.ones(nslots, dtype=np.float32)
                if n > 0:
                    d_pad[:n] = (dst_s[a:a + n] - off).astype(np.int16)
                    d_pad[n:] = d_pad[n - 1] if n > 0 else 0
                    s_pad[:n] = srcl_s[a:a + n]
                    c_pad[:n] = scale4[a:a + n]
                sb = int(cum[w])           # stream batch base
                IDX[c, sb * P:(sb + nbw) * P] = d_pad
                # metadata indexed by (stream, stream-batch): L block then H block
                cb = sb + (NBL if h == 1 else 0)
                # slot i -> (partition i%P, batch i//P)
                SRCL[c, :, cb:cb + nbw] = s_pad.reshape(nbw, P).T
                SC4[c, :, cb:cb + nbw] = c_pad.reshape(nbw, P).T

    XO = np.zeros((N_CORES, npc_pad, F), dtype=np.float32)
    DG2 = np.zeros((N_CORES, P, NW), dtype=np.float32)
    DEGROW = np.zeros((N_CORES, 1, npc_pad), dtype=np.float32)
    nf = np.asarray(node_features, dtype=np.float32)
    for c in range(N_CORES):
        XO[c, :npc] = nf[c * npc:(c + 1) * npc]
        dpad = np.zeros(npc_pad, dtype=np.float32)
        dpad[:npc] = 0.5 * deg[c * npc:(c + 1) * npc]
        DG2[c] = dpad.reshape(NW, P).T
        DEGROW[c, 0] = dpad

    IDXLw = np.stack([_wrap16(IDXL[c]) for c in range(N_CORES)]) if NBL else \
        np.zeros((N_CORES, P, 0), np.int16)
    IDXHw = np.stack([_wrap16(IDXH[c]) for c in range(N_CORES)]) if NBH else \
        np.zeros((N_CORES, P, 0), np.int16)

    return dict(IDXL=IDXLw, IDXH=IDXHw, SRCL=SRCL, SC4=SC4, XO=XO, DG2=DG2,
                DEGROW=DEGROW, NW=NW, NBL=NBL, NBH=NBH, NBtot=NBtot,
                nbL=nbL, nbH=nbH, cumL=cumL, cumH=cumH, base=base,
                npc=npc, npc_pad=npc_pad)


def _build_nc(hd, n_nodes, has_b):
    import concourse.bass as bass
    import concourse.bacc as bacc
    import concourse.mybir as mybir
    import concourse.tile as tile

    NW, NBL, NBH, NBtot = hd["NW"], hd["NBL"], hd["NBH"], hd["NBtot"]
    nbL, nbH = hd["nbL"], hd["nbH"]
    cumL, cumH, base = hd["cumL"], hd["cumH"], hd["base"]
    npc_pad = hd["npc_pad"]

    fp32 = mybir.dt.float32
    nc = bacc.Bacc("TRN2", target_bir_lowering=False, debug=False,
                   num_swdge_queues=4)

    nf_d = nc.dram_tensor("NF", [n_nodes, F], fp32, kind="ExternalInput")
    xo_d = nc.dram_tensor("XO", [npc_pad, F], fp32, kind="ExternalInput")
    il_d = nc.dram_tensor("IDXL", [P, NBL * 8], mybir.dt.int16, kind="ExternalInput")
    ih_d = nc.dram_tensor("IDXH", [P, NBH * 8], mybir.dt.int16, kind="ExternalInput")
    srcl_d = nc.dram_tensor("SRCL", [P, NBtot], fp32, kind="ExternalInput")
    sc4_d = nc.dram_tensor("SC4", [P, NBtot], fp32, kind="ExternalInput")
    dg2_d = nc.dram_tensor("DG2", [P, NW], fp32, kind="ExternalInput")
    iota_d = nc.dram_tensor("IOTA8", [P, 8 * P], fp32, kind="ExternalInput")
    ident_d = nc.dram_tensor("IDENT", [P, P], fp32, kind="ExternalInput")
    w_d = nc.dram_tensor("WM", [F, H], fp32, kind="ExternalInput")
    gm_d = nc.dram_tensor("GCOL", [P, 1], fp32, kind="ExternalInput")
    bt_d = nc.dram_tensor("BTCOL", [P, 1], fp32, kind="ExternalInput")
    mm_d = nc.dram_tensor("MMCOL", [P, 1], fp32, kind="ExternalInput")
    mv_d = nc.dram_tensor("MVCOL", [P, 1], fp32, kind="ExternalInput")
    if has_b:
        brow_d = nc.dram_tensor("BROW", [1, H], fp32, kind="ExternalInput")
        degrow_d = nc.dram_tensor("DEGROW", [1, npc_pad], fp32, kind="ExternalInput")
    out_d = nc.dram_tensor("OUT_T", [P, npc_pad], fp32, kind="ExternalOutput")

    with tile.TileContext(nc) as tc:
        with (
            tc.tile_pool(name="meta", bufs=1) as meta,
            tc.tile_pool(name="gl", bufs=7) as glpool,
            tc.tile_pool(name="gh", bufs=5) as ghpool,
            tc.tile_pool(name="o", bufs=4) as opool,
            tc.tile_pool(name="x", bufs=2) as xpool,
            tc.tile_pool(name="z", bufs=2) as zpool,
            tc.tile_pool(name="slab", bufs=1) as slab,
            tc.tile_pool(name="psz", bufs=2, space="PSUM") as psZ,
            tc.tile_pool(name="psz2", bufs=2, space="PSUM") as psZ2,
            tc.tile_pool(name="psnb", bufs=1, space="PSUM") as psNB,
            tc.tile_pool(name="psx", bufs=1, space="PSUM") as psX,
        ):
            il_sb = meta.tile([P, max(NBL, 1) * 8], mybir.dt.int16)
            ih_sb = meta.tile([P, max(NBH, 1) * 8], mybir.dt.int16)
            srcl_sb = meta.tile([P, NBtot], fp32)
            sc4_sb = meta.tile([P, NBtot], fp32)
            scl_sb = meta.tile([P, NBtot], fp32)
            iota8_sb = meta.tile([P, 8 * P], fp32)
            ident_sb = meta.tile([P, P], fp32)
            w_sb = meta.tile([F, H], fp32)
            dg2_sb = meta.tile([P, NW], fp32)
            gm_sb = meta.tile([P, 1], fp32)
            bt_sb = meta.tile([P, 1], fp32)
            mm_sb = meta.tile([P, 1], fp32)
            mv_sb = meta.tile([P, 1], fp32)
            rs_sb = meta.tile([P, 1], fp32)
            gp_sb = meta.tile([P, 1], fp32)
            bb_sb = meta.tile([P, 1], fp32)

            if NBL:
                nc.sync.dma_start(il_sb[:, :NBL * 8], il_d[:])
            if NBH:
                nc.sync.dma_start(ih_sb[:, :NBH * 8], ih_d[:])
            nc.sync.dma_start(srcl_sb[:], srcl_d[:])
            nc.sync.dma_start(sc4_sb[:], sc4_d[:])
            nc.sync.dma_start(iota8_sb[:], iota_d[:])
            nc.sync.dma_start(w_sb[:], w_d[:])
            nc.sync.dma_start(dg2_sb[:], dg2_d[:])
            nc.sync.dma_start(gm_sb[:], gm_d[:])
            nc.sync.dma_start(bt_sb[:], bt_d[:])
            nc.sync.dma_start(mm_sb[:], mm_d[:])
            nc.sync.dma_start(mv_sb[:], mv_d[:])
            nc.sync.dma_start(ident_sb[:], ident_d[:])

            # scale' = rsqrt(4*deg_s*deg_d) = 0.5*isd_s*isd_d (Sqrt + exact reciprocal)
            nc.scalar.activation(scl_sb[:], sc4_sb[:], mybir.ActivationFunctionType.Sqrt)
            nc.vector.reciprocal(scl_sb[:], scl_sb[:])

            # BN: g' = gamma * rsqrt(var+eps);  bb = beta - mean*g'
            nc.vector.tensor_scalar(out=rs_sb[:], in0=mv_sb[:], scalar1=BN_EPS,
                                    scalar2=None, op0=mybir.AluOpType.add)
            nc.scalar.activation(rs_sb[:], rs_sb[:], mybir.ActivationFunctionType.Sqrt)
            nc.vector.reciprocal(rs_sb[:], rs_sb[:])
            nc.vector.tensor_tensor(out=gp_sb[:], in0=gm_sb[:], in1=rs_sb[:],
                                    op=mybir.AluOpType.mult)
            nc.vector.tensor_tensor(out=bb_sb[:], in0=mm_sb[:], in1=gp_sb[:],
                                    op=mybir.AluOpType.mult)
            nc.vector.tensor_tensor(out=bb_sb[:], in0=bt_sb[:], in1=bb_sb[:],
                                    op=mybir.AluOpType.subtract)

            if has_b:
                brow_sb = meta.tile([1, H], fp32)
                degrow_sb = meta.tile([1, npc_pad], fp32)
                nc.sync.dma_start(brow_sb[:], brow_d[:])
                nc.sync.dma_start(degrow_sb[:], degrow_d[:])

            outT_sb = slab.tile([P, npc_pad], fp32)

            # ---- gather machinery: two streams (low/high table halves) ----
            streams = {
                "L": dict(nb=NBL, idx=il_sb, view=nf_d[0:min(SPLIT, n_nodes)],
                          pool=glpool, tiles={}),
                "H": dict(nb=NBH, idx=ih_sb, view=(nf_d[SPLIT:n_nodes]
                                                   if n_nodes > SPLIT else None),
                          pool=ghpool, tiles={}),
            }

            # emit all gathers up front, in consumption (first-use) order
            chunk_order, seen = [], set()
            for w in range(NW):
                for k in range(int(nbL[w]) + int(nbH[w])):
                    if k < int(nbL[w]):
                        key = ("L", (int(cumL[w]) + k) // CHB)
                    else:
                        key = ("H", (int(cumH[w]) + k - int(nbL[w])) // CHB)
                    if key not in seen:
                        seen.add(key)
                        chunk_order.append(key)
            for sname, ci in chunk_order:
                st = streams[sname]
                c0, c1 = ci * CHB, min((ci + 1) * CHB, st["nb"])
                nbc = c1 - c0
                gt = st["pool"].tile([P, nbc, F], fp32, tag="g" + sname)
                nidx = nbc * P
                nc.gpsimd.dma_gather(
                    gt[:], st["view"], st["idx"][:, c0 * 8:c1 * 8],
                    nidx, nidx, F, single_packet=False, queue_num=0)
                st["tiles"][ci] = (c0, gt)

            def gslice(s, j):
                c0, gt = streams[s]["tiles"][j // CHB]
                return gt[:, j - c0, :]

            # ---- batched one-hot machinery ----
            # per stream, groups of GRP stream-batches; O8 = is_equal(iota8,
            # srcl bcast), then scaled by scl: 2/3 of groups on DVE (batched
            # TT), 1/3 on ACT (per-batch, per-partition scale).
            GRP = 8
            ogroups = {"L": {}, "H": {}}

            def ensure_group(sname, g):
                grp = ogroups[sname]
                if g in grp:
                    return
                nb_s = streams[sname]["nb"]
                off = 0 if sname == "L" else NBL
                g0 = g * GRP
                m = min(GRP, nb_s - g0)
                o8 = opool.tile([P, m * P], fp32, tag="o8")
                nc.vector.tensor_tensor(
                    out=o8[:], in0=iota8_sb[:, :m * P],
                    in1=srcl_sb[:, off + g0:off + g0 + m].to_broadcast([P, m, P]),
                    op=mybir.AluOpType.is_equal)
                o8s = opool.tile([P, m * P], fp32, tag="o8s")
                if (g0 // GRP) % 3 == 0:
                    for k in range(m):
                        nc.scalar.activation(
                            o8s[:, k * P:(k + 1) * P], o8[:, k * P:(k + 1) * P],
                            mybir.ActivationFunctionType.Identity,
                            scale=scl_sb[:, off + g0 + k:off + g0 + k + 1])
                else:
                    nc.vector.tensor_tensor(
                        out=o8s[:], in0=o8[:],
                        in1=scl_sb[:, off + g0:off + g0 + m].to_broadcast([P, m, P]),
                        op=mybir.AluOpType.mult)
                grp[g] = o8s

            def oslice(sname, sj):
                ensure_group(sname, sj // GRP)
                o8s = ogroups[sname][sj // GRP]
                k = sj % GRP
                return o8s[:, k * P:(k + 1) * P]

            # ---- main window loop ----
            out_dma_step = max(1, NW // 4)
            for w in range(NW):
                nl, nh = int(nbL[w]), int(nbH[w])
                nbw = nl + nh
                zt = None
                if nbw > 0:
                    # alternate PSUM banks to break the accumulation RAW chain
                    psa = psZ.tile([P, P], fp32)
                    psb = psZ2.tile([P, P], fp32, name=f"psb{w}", tag="psb") if nbw > 1 else None
                    na = (nbw + 1) // 2
                    nb_ = nbw // 2
                    ia = ib = 0
                    for k in range(nbw):
                        if k < nl:
                            sname, sj = "L", int(cumL[w]) + k
                        else:
                            sname, sj = "H", int(cumH[w]) + (k - nl)
                        g_ap = gslice(sname, sj)
                        o_ap = oslice(sname, sj)
                        if k % 2 == 0:
                            nc.tensor.matmul(psa[:], lhsT=g_ap, rhs=o_ap,
                                             start=(ia == 0), stop=(ia == na - 1))
                            ia += 1
                        else:
                            nc.tensor.matmul(psb[:], lhsT=g_ap, rhs=o_ap,
                                             start=(ib == 0), stop=(ib == nb_ - 1))
                            ib += 1
                    zt = zpool.tile([P, P], fp32, tag="z")
                    nc.scalar.copy(zt[:], psa[:])
                    if psb is not None:
                        nc.vector.tensor_tensor(out=zt[:], in0=zt[:], in1=psb[:],
                                                op=mybir.AluOpType.add)

                # source term: (0.5*deg * X_own) transposed
                xt = xpool.tile([P, P], fp32, tag="xt")
                nc.sync.dma_start(xt[:], xo_d[w * P:(w + 1) * P, :])
                xs = xpool.tile([P, P], fp32, tag="xs")
                nc.scalar.activation(xs[:], xt[:],
                                     mybir.ActivationFunctionType.Identity,
                                     scale=dg2_sb[:, w:w + 1])
                pxt = psX.tile([P, P], fp32)
                nc.tensor.transpose(pxt[:], xs[:], ident_sb[:])
                xT = xpool.tile([P, P], fp32, tag="xT")
                nc.scalar.copy(xT[:], pxt[:])

                psnb = psNB.tile([P, P], fp32)
                first = True
                if zt is not None:
                    nc.tensor.matmul(psnb[:], lhsT=w_sb[:], rhs=zt[:],
                                     start=True, stop=False)
                    first = False
                nc.tensor.matmul(psnb[:], lhsT=w_sb[:], rhs=xT[:],
                                 start=first, stop=not has_b)
                if has_b:
                    nc.tensor.matmul(psnb[:], lhsT=brow_sb[:],
                                     rhs=degrow_sb[:, w * P:(w + 1) * P],
                                     start=False, stop=True)

                # BN affine (per-partition in feature-major layout)
                nc.scalar.activation(
                    outT_sb[:, w * P:(w + 1) * P], psnb[:],
                    mybir.ActivationFunctionType.Identity,
                    bias=bb_sb[:], scale=gp_sb[:],
                )

                if (w + 1) % out_dma_step == 0 or w == NW - 1:
                    lo = (w // out_dma_step) * out_dma_step
                    nc.sync.dma_start(out_d[:, lo * P:(w + 1) * P],
                                      outT_sb[:, lo * P:(w + 1) * P])

    # SWDGE queue ownership: each DMASW sem lane is owned by one queue, so
    # set queue_num = lane % num_queues after Tile assigned lanes.
    from concourse.tile_scheduler import PROC_NAME_TO_IDX
    idx_to_proc = {v: k for k, v in PROC_NAME_TO_IDX.items()}
    for bb in nc.main_func.blocks:
        for ins in bb.instructions:
            if isinstance(ins, mybir.InstDMAGatherAnt):
                proc = idx_to_proc.get(ins.bass_scheduled_proc, "")
                if proc.startswith("DMASW"):
                    ins.queue_num = int(proc[5:]) % 4

    nc.compile()
    return nc


def _prepare(edge_pairs, node_features, W, b, gamma, beta, moving_mean, moving_var):
    n_nodes, _ = node_features.shape
    hd = _build_host_data(edge_pairs, node_features)
    has_b = bool(np.any(np.asarray(b) != 0))

    key = (n_nodes, node_features.shape[1], hd["NBtot"],
           tuple(hd["nbL"].tolist()), tuple(hd["nbH"].tolist()), has_b)
    if key not in _CACHE:
        _CACHE.clear()
        _CACHE[key] = _build_nc(hd, n_nodes, has_b)
    nc = _CACHE[key]

    nf = np.ascontiguousarray(np.asarray(node_features, dtype=np.float32))
    iota8 = np.tile(np.arange(P, dtype=np.float32), (P, 8)).reshape(P, 8 * P).copy()
    in_maps = []
    for c in range(N_CORES):
        m = {
            "NF": nf,
            "XO": np.ascontiguousarray(hd["XO"][c]),
            "IDXL": np.ascontiguousarray(hd["IDXL"][c]),
            "IDXH": np.ascontiguousarray(hd["IDXH"][c]),
            "SRCL": np.ascontiguousarray(hd["SRCL"][c]),
            "SC4": np.ascontiguousarray(hd["SC4"][c]),
            "DG2": np.ascontiguousarray(hd["DG2"][c]),
            "IOTA8": iota8,
            "IDENT": np.eye(P, dtype=np.float32),
            "WM": np.ascontiguousarray(np.asarray(W, dtype=np.float32)),
            "GCOL": np.asarray(gamma, np.float32).reshape(P, 1).copy(),
            "BTCOL": np.asarray(beta, np.float32).reshape(P, 1).copy(),
            "MMCOL": np.asarray(moving_mean, np.float32).reshape(P, 1).copy(),
            "MVCOL": np.asarray(moving_var, np.float32).reshape(P, 1).copy(),
        }
        if has_b:
            m["BROW"] = np.asarray(b, np.float32).reshape(1, H).copy()
            m["DEGROW"] = np.ascontiguousarray(hd["DEGROW"][c])
        in_maps.append(m)
    return nc, in_maps, hd


def _run(inputs, trace=False):
    from concourse.bass_utils import run_bass_kernel_spmd

    nc, in_maps, hd = _prepare(**inputs)
    res = run_bass_kernel_spmd(nc, in_maps, core_ids=list(range(N_CORES)),
                               trace=trace)
    npc = hd["npc"]
    out = np.empty((npc * N_CORES, H), dtype=np.float32)
    for c in range(N_CORES):
        out[c * npc:(c + 1) * npc] = res.results[c]["OUT_T"].T[:npc]
    return out, res


def kernel(**inputs):
    out, _ = _run(inputs, trace=False)
    return out


def run_traced(**inputs):
    return _run(inputs, trace=True)
